# revision 19
# baseline (speedup 1.0000x reference)
"""Bass/Trainium2 kernel for nn_EpisodeMultiheadAttentionBlock.

Reference computation (B=4, L=4096, E=256, Q=2048):
    x  = key[:, -Q:]
    q  = rope(x @ Wq.T + bq, idx[:, -Q:]);  k = rope(key @ Wk.T + bk, idx)
    v  = key @ Wv.T + bv
    s  = (q / sqrt(E)) @ k.T  with causal mask (bottom Q rows of triu)
    w  = softmax(s)                            -> output 2
    y  = (w @ v) @ Wo.T + bo
    r  = sigmoid(x@Wxr.T + y@Wyr.T); z = sigmoid(x@Wxz.T + y@Wyz.T)
    hh = tanh((r*x)@Wxg.T + y@Wyg.T)
    out = (1-z)*x + z*hh                       -> output 1

Sharding: 8 cores = 4 batches x 2 query-interleavings. Core (b, h) owns
query 128-row blocks g = 2m+h (m=0..7) of batch b. The interleaving makes
the causal key extent per local block m uniform across cores: E(m) = 18+2m
key blocks (even-parity cores carry one fully-masked pad block), so one
SPMD program serves all 8 cores; the parity-dependent boundary mask is a
tiny per-core input added into the score PSUM via an identity matmul.

Layouts: projections/gating run in "T layout" (feature dim on partitions)
so every matmul's operands arrive in the orientation the next matmul
consumes. RoPE's pair rotation becomes a between-subtile elementwise op by
permuting the feature dim of Wq/Wk to [evens | odds] host-side (the
permutation cancels inside q.k). Scores are computed naturally
([query-part, key-free]) so the softmax row-sum falls out of the scalar
engine's exp accum_out and w DMAs straight out; the attention matmul's
j-on-partitions operand is built with PE transposes of the exp'd tiles.
"""

import sys

sys.path.insert(0, "/opt/trn_rl_repo")

import numpy as np
import ml_dtypes

import concourse.bass as bass
import concourse.tile as tile
from concourse import bacc, mybir
from concourse.bass_utils import run_bass_kernel_spmd
from concourse.masks import make_identity

F32 = mybir.dt.float32
F32R = mybir.dt.float32r
BF16 = mybir.dt.bfloat16
BF16_NP = np.dtype(ml_dtypes.bfloat16)

B, L, E, Q = 4, 4096, 256, 2048
P = 128
NM = 8              # query 128-blocks per core
NEG = -30000.0      # additive mask; exp(NEG/16) underflows to exactly 0.0
SCALE = 1.0 / 16.0  # 1/sqrt(E)

IdF = mybir.ActivationFunctionType.Identity
ExpF = mybir.ActivationFunctionType.Exp
SigF = mybir.ActivationFunctionType.Sigmoid
TanhF = mybir.ActivationFunctionType.Tanh

_CACHE = {}


def _ext_blocks(m):
    """Causal key extent (in 128-col blocks) for local query block m."""
    return 18 + 2 * m


def build_nc():
    nc = bacc.Bacc("TRN2", target_bir_lowering=False, debug=False,
                   enable_asserts=False, num_devices=8)

    io = {}
    io["keyT"] = nc.dram_tensor("keyT", [P, 2, L], F32,
                                kind="ExternalInput").ap()
    io["xT_in"] = nc.dram_tensor("xT_in", [P, 2, Q // 2], F32,
                                 kind="ExternalInput").ap()
    io["cosT"] = nc.dram_tensor("cosT", [P, L], BF16,
                                kind="ExternalInput").ap()
    io["sinT"] = nc.dram_tensor("sinT", [P, L], BF16,
                                kind="ExternalInput").ap()
    io["qcs_in"] = nc.dram_tensor("qcs_in", [P, 2, Q // 2], BF16,
                                  kind="ExternalInput").ap()
    io["wts"] = nc.dram_tensor("wts", [P, 2, 10, E], F32,
                               kind="ExternalInput").ap()
    io["mtail"] = nc.dram_tensor("mtail", [P, 2 * P], BF16,
                                 kind="ExternalInput").ap()
    io["biasv"] = nc.dram_tensor("biasv", [P, 2, 6], F32,
                                 kind="ExternalInput").ap()
    io["bvbc"] = nc.dram_tensor("bvbc", [P, E], F32,
                                kind="ExternalInput").ap()
    io["w_out"] = nc.dram_tensor("w_out", [NM, P, L], F32,
                                 kind="ExternalOutput").ap()
    io["o_out"] = nc.dram_tensor("o_out", [NM, P, E], F32,
                                 kind="ExternalOutput").ap()

    with tile.TileContext(nc) as tc:
        _emit(nc, tc, io)
    nc.compile()
    return nc


def _emit(nc, tc, io):
    from contextlib import ExitStack

    with ExitStack() as ctx:
        const = ctx.enter_context(tc.tile_pool(name="const", bufs=1))
        big = ctx.enter_context(tc.tile_pool(name="big", bufs=1))
        # single PSUM pool layout (8 banks total):
        #   mmps [P,1024] x2bufs = 4 banks, tpps [P,512] x2 = 2,
        #   atps [P,256] x2 = 2
        mmps = ctx.enter_context(tc.tile_pool(name="mmps", bufs=2,
                                              space="PSUM"))
        tpps = ctx.enter_context(tc.tile_pool(name="tpps", bufs=2,
                                              space="PSUM"))
        atps = ctx.enter_context(tc.tile_pool(name="atps", bufs=2,
                                              space="PSUM"))

        def mm_tile(cols=1024):
            return mmps.tile([P, 1024], F32, name="mmt", tag="mm")[:, :cols]

        # ---- q path first: it gates the first score matmuls ----------
        wnames = ["wq", "wk", "wv", "wo", "wxr", "wyr", "wxz", "wyz",
                  "wxg", "wyg"]
        biasv = const.tile([P, 2, 6], F32)
        nc.sync.dma_start(biasv[:], io["biasv"])
        wts = const.tile([P, 2, 10, E], F32R)
        v = big.tile([P, L // P, E], F32R)         # 32KB/part
        xTr = big.tile([P, 2, Q // 2], F32R)       # 8KB (f32r-rounded x)

        with tc.tile_pool(name="rotp", bufs=1) as rotp:
            kTrot = rotp.tile([P, 2, L], BF16)     # 16KB
            qTrot = rotp.tile([P, 2, Q // 2], BF16)

            with tc.tile_pool(name="stgp", bufs=1) as stgp, \
                 tc.tile_pool(name="keyp", bufs=1) as keyp, \
                 tc.tile_pool(name="csp", bufs=1) as csp, \
                 tc.tile_pool(name="ropet", bufs=2) as ropet:
                stg = stgp.tile([P, 2, 10, E], F32, name="stg", tag="stg")
                nc.sync.dma_start(stg[:, :, :3, :], io["wts"][:, :, :3, :])
                nc.vector.tensor_copy(wts[:, :, :3, :], stg[:, :, :3, :])
                wt = {n: wts[:, :, i, :] for i, n in enumerate(wnames)}
                xs = keyp.tile([P, 2, Q // 2], F32, name="xs", tag="ks",
                               bufs=2)
                nc.sync.dma_start(xs[:], io["xT_in"])
                nc.vector.tensor_copy(xTr[:], xs[:])
                qcs = csp.tile([P, 2, Q // 2], BF16)
                nc.sync.dma_start(qcs[:], io["qcs_in"])

                def project_rot(dst, src_rhs, wn_, cos_ap, sin_ap, bcol,
                                n, cb=None, CW=1024):
                    """dst[:,0,:] = a*c - b*s ; dst[:,1,:] = a*s + b*c
                    with (a,b) = halves of (W @ src + bias)."""
                    for ci, c0 in enumerate(range(0, n, CW)):
                        cw = min(CW, n - c0)
                        pa = mm_tile(cw)
                        pb = mm_tile(cw)
                        for dh, ps in ((0, pa), (1, pb)):
                            for s in range(2):
                                for q0 in range(0, cw, 512):
                                    qw = min(512, cw - q0)
                                    nc.tensor.matmul(
                                        ps[:, q0:q0 + qw],
                                        wt[wn_][:, s, dh * P:(dh + 1) * P],
                                        src_rhs[:, s, c0 + q0:c0 + q0 + qw],
                                        start=(s == 0), stop=(s == 1))
                        ab = ropet.tile([P, 2, 1024], BF16,
                                        name="ab", tag="ab")[:, :, :cw]
                        nc.scalar.activation(ab[:, 0, :], pa[:], IdF,
                                             bias=biasv[:, 0, bcol:bcol + 1])
                        nc.scalar.activation(ab[:, 1, :], pb[:], IdF,
                                             bias=biasv[:, 1, bcol:bcol + 1])
                        c_ap = cos_ap[:, c0:c0 + cw]
                        s_ap = sin_ap[:, c0:c0 + cw]
                        t1 = ropet.tile([P, 1024], BF16, name="t1",
                                        tag="t1", bufs=1)[:, :cw]
                        t2 = ropet.tile([P, 1024], BF16, name="t2",
                                        tag="t2", bufs=1)[:, :cw]
                        nc.vector.tensor_mul(t1[:], ab[:, 0, :], c_ap)
                        nc.vector.tensor_mul(t2[:], ab[:, 1, :], s_ap)
                        nc.vector.tensor_tensor(dst[:, 0, c0:c0 + cw],
                                                t1[:], t2[:],
                                                mybir.AluOpType.subtract)
                        t3 = ropet.tile([P, 1024], BF16, name="t3",
                                        tag="t3", bufs=2)[:, :cw]
                        t4 = ropet.tile([P, 1024], BF16, name="t4",
                                        tag="t4", bufs=2)[:, :cw]
                        nc.gpsimd.tensor_mul(t3[:], ab[:, 0, :], s_ap)
                        nc.vector.tensor_mul(t4[:], ab[:, 1, :], c_ap)
                        nc.vector.tensor_tensor(dst[:, 1, c0:c0 + cw],
                                                t3[:], t4[:],
                                                mybir.AluOpType.add)
                        if cb is not None:
                            cb(ci)

                # q projection + rope (only needs xs/wq/qcs -- starts fast)
                project_rot(qTrot, xTr, "wq", qcs[:, 0, :], qcs[:, 1, :],
                            0, Q // 2)

                # keyT load + round, chunk 0 first
                keyT = keyp.tile([P, 2, L], F32R)  # 32KB
                for ci, c0 in enumerate(range(0, L, 1024)):
                    ks = keyp.tile([P, 2, 1024], F32, name="ks", tag="ks",
                                   bufs=2)
                    nc.sync.dma_start(ks[:], io["keyT"][:, :, c0:c0 + 1024])
                    eng = nc.vector if ci % 2 == 0 else nc.gpsimd
                    eng.tensor_copy(keyT[:, :, c0:c0 + 1024], ks[:])
                cosT = csp.tile([P, L], BF16)
                sinT = csp.tile([P, L], BF16)
                nc.sync.dma_start(cosT[:], io["cosT"])
                nc.sync.dma_start(sinT[:], io["sinT"])

                # k projection + rope, with v-projection interleaved
                # (v group j4 covers keyT cols [512*j4, 512*j4+512) --
                #  exactly inside rope chunk ci = j4 // 2)
                def v_groups(ci):
                    for j4 in (2 * ci, 2 * ci + 1):
                        vps = mm_tile().rearrange("p (j d) -> p j d", d=E)
                        for jj in range(4):
                            J = j4 * 4 + jj
                            for s in range(2):
                                nc.tensor.matmul(
                                    vps[:, jj, :],
                                    keyT[:, s, J * P:(J + 1) * P],
                                    wt["wv"][:, s, :],
                                    start=(s == 0), stop=(s == 1))
                        dst = v[:, j4 * 4:(j4 + 1) * 4, :]
                        if j4 % 2 == 0:
                            nc.vector.tensor_copy(dst, vps[:])
                        else:
                            nc.scalar.copy(dst, vps[:])

                project_rot(kTrot, keyT, "wk", cosT, sinT, 1, L,
                            cb=v_groups)

                # remaining consts + gating weights (off critical path)
                ident = const.tile([P, P], F32)
                make_identity(nc, ident)
                identr = const.tile([P, P], F32R)
                nc.vector.tensor_copy(identr[:], ident[:])
                identb = const.tile([P, P], BF16)
                make_identity(nc, identb)
                mtail = const.tile([P, 2 * P], BF16)
                nc.sync.dma_start(mtail[:], io["mtail"])
                bvbc = const.tile([P, E], F32)
                nc.sync.dma_start(bvbc[:], io["bvbc"])
                nc.sync.dma_start(stg[:, :, 3:, :], io["wts"][:, :, 3:, :])
                nc.scalar.copy(wts[:, :, 3:, :], stg[:, :, 3:, :])

            # ---- main attention loop ---------------------------------
            with tc.tile_pool(name="expp", bufs=8) as expp, \
                 tc.tile_pool(name="wnp", bufs=4) as wnp, \
                 tc.tile_pool(name="wtp", bufs=6) as wtp, \
                 tc.tile_pool(name="zp", bufs=1) as zp, \
                 tc.tile_pool(name="gatep", bufs=1) as gatep, \
                 tc.tile_pool(name="smallp", bufs=2) as smallp:
                zeros = zp.tile([P, 1792], F32)
                nc.vector.memset(zeros[:], 0.0)
                attn = gatep.tile([P, NM, E], F32, name="attn")
                recips = gatep.tile([P, NM], F32, name="recips")

                def mm_Th(dst_sb, terms, bcol, act):
                    """dst[:,dh,:] = act(sum W.T-lhsT @ rhs[roff:] + b).
                    For Sigmoid, biasv[bcol] holds the NEGATED bias and we
                    compute 1/(1+exp(-t)) via the exp table set (no ACT
                    table switch; Tanh shares exp's set)."""
                    for dh in range(2):
                        pr = atps.tile([P, 512], F32, name="pr",
                                       tag="aps")
                        ng = len(terms)
                        for gi, (wn_, rhs, roff) in enumerate(terms):
                            for s in range(2):
                                nc.tensor.matmul(
                                    pr[:],
                                    wt[wn_][:, s, dh * P:(dh + 1) * P],
                                    rhs[:, s, roff:roff + 512],
                                    start=(gi == 0 and s == 0),
                                    stop=(gi == ng - 1 and s == 1))
                        if act is SigF:
                            tmp = gatep.tile([P, 512], F32, name="sgt",
                                             tag="sgt", bufs=2)
                            nc.scalar.activation(tmp[:], pr[:], ExpF,
                                                 bias=biasv[:, dh,
                                                            bcol:bcol + 1],
                                                 scale=-1.0)
                            nc.vector.tensor_scalar_add(tmp[:], tmp[:], 1.0)
                            nc.vector.reciprocal(dst_sb[:, dh, :], tmp[:])
                        else:
                            nc.scalar.activation(dst_sb[:, dh, :], pr[:],
                                                 act,
                                                 bias=biasv[:, dh,
                                                            bcol:bcol + 1])

                def emit_gate_half(half):
                    """GRU gating for query blocks [4*half, 4*half+4)."""
                    i0 = half * 512
                    attnT = gatep.tile([P, 2, 512], F32R, name="attnT",
                                       tag="attnT")
                    for dh in range(2):
                        tp = tpps.tile([P, 512], F32, name="tpg", tag="tp")
                        for mm in range(4):
                            m_ = half * 4 + mm
                            nc.tensor.transpose(
                                tp[:, mm * P:(mm + 1) * P],
                                attn[:, m_, dh * P:(dh + 1) * P],
                                ident[:])
                        if dh == 0:
                            nc.vector.tensor_copy(attnT[:, dh, :], tp[:])
                        else:
                            nc.scalar.copy(attnT[:, dh, :], tp[:])
                    yT = gatep.tile([P, 2, 512], F32R, name="yT", tag="yT")
                    mm_Th(yT, [("wo", attnT, 0)], 5, IdF)
                    rT = gatep.tile([P, 2, 512], F32, name="rT", tag="rT")
                    mm_Th(rT, [("wxr", xTr, i0), ("wyr", yT, 0)], 2, SigF)
                    zT = gatep.tile([P, 2, 512], F32, name="zT", tag="zT")
                    mm_Th(zT, [("wxz", xTr, i0), ("wyz", yT, 0)], 3, SigF)
                    xf = xTr.bitcast(F32)[:, :, i0:i0 + 512]
                    gx = gatep.tile([P, 2, 512], F32R, name="gx", tag="gx")
                    for dh in range(2):
                        nc.vector.tensor_mul(gx[:, dh, :], rT[:, dh, :],
                                             xf[:, dh, :])
                    hT = gatep.tile([P, 2, 512], F32, name="hT", tag="hT")
                    mm_Th(hT, [("wyg", yT, 0), ("wxg", gx, 0)], 4, TanhF)
                    outT = gatep.tile([P, 2, 512], F32, name="outT",
                                      tag="outT")
                    for dh in range(2):
                        d1 = gatep.tile([P, 512], F32, name="d1", tag="d1",
                                        bufs=2)
                        nc.vector.tensor_tensor(d1[:], hT[:, dh, :],
                                                xf[:, dh, :],
                                                mybir.AluOpType.subtract)
                        nc.vector.tensor_mul(d1[:], zT[:, dh, :], d1[:])
                        nc.vector.tensor_add(outT[:, dh, :], xf[:, dh, :],
                                             d1[:])
                    onat = gatep.tile([P, 4, E], F32, name="onat",
                                      tag="onat")
                    for dh in range(2):
                        tp = tpps.tile([P, 512], F32, name="tpo", tag="tp")
                        for mm in range(4):
                            nc.tensor.transpose(
                                tp[:, mm * P:(mm + 1) * P],
                                outT[:, dh, mm * P:(mm + 1) * P],
                                ident[:])
                        for mm in range(4):
                            dst = onat[:, mm, dh * P:(dh + 1) * P]
                            src = tp[:, mm * P:(mm + 1) * P]
                            if dh == 0:
                                nc.vector.tensor_copy(dst, src)
                            else:
                                nc.scalar.copy(dst, src)
                    nc.sync.dma_start(
                        io["o_out"].rearrange("m p e -> p m e")
                        [:, half * 4:half * 4 + 4, :], onat[:])

                for mi, m in enumerate([0, 1, 7, 6, 5, 4, 3, 2]):
                    EB = _ext_blocks(m)
                    ext = EB * P
                    nch = (ext + 1023) // 1024
                    parts = smallp.tile([P, 4], F32, name="parts",
                                        tag="parts")
                    aps = atps.tile([P, E], F32, name="aps", tag="aps")
                    exp_tiles = []
                    jdone = 0
                    for c in range(nch):
                        c0 = c * 1024
                        cw = min(1024, ext - c0)
                        sps = mm_tile(cw)
                        for q0 in range(0, cw, 512):
                            qw = min(512, cw - q0)
                            last = (c == nch - 1) and (q0 + qw == cw)
                            for s in range(2):
                                nc.tensor.matmul(
                                    sps[:, q0:q0 + qw],
                                    qTrot[:, s, m * P:(m + 1) * P],
                                    kTrot[:, s, c0 + q0:c0 + q0 + qw],
                                    start=(s == 0),
                                    stop=(s == 1) and not last)
                            if last:
                                nc.tensor.matmul(
                                    sps[:, cw - 2 * P:cw], identb[:],
                                    mtail[:], start=False, stop=True,
                                    skip_group_check=True)
                        ex = expp.tile([P, 1024], F32R, name="ex",
                                       tag="ex")[:, :cw]
                        nc.scalar.activation(ex[:], sps[:], ExpF, bias=0.0,
                                             scale=SCALE,
                                             accum_out=parts[:, c:c + 1])
                        exp_tiles.append((ex, c0, cw))
                        for g0 in range(0, cw, 512):
                            gw = min(512, cw - g0)
                            nblk = gw // P
                            tp = tpps.tile([P, 512], F32R, name="tp",
                                           tag="tp")[:, :gw]
                            for jj in range(nblk):
                                nc.tensor.transpose(
                                    tp[:, jj * P:(jj + 1) * P],
                                    ex[:, g0 + jj * P:g0 + (jj + 1) * P],
                                    identr[:])
                            wTs = wtp.tile([P, 512], F32R, name="wTs",
                                           tag="wTs")[:, :gw]
                            if (jdone // 4) % 2 == 0:
                                nc.vector.tensor_copy(wTs[:], tp[:])
                            else:
                                nc.scalar.copy(wTs[:], tp[:])
                            for jj in range(nblk):
                                J = jdone + jj
                                nc.tensor.matmul(
                                    aps[:],
                                    wTs[:, jj * P:(jj + 1) * P],
                                    v[:, J, :],
                                    start=(J == 0), stop=(J == EB - 1))
                            jdone += nblk
                    sig = smallp.tile([P, 1], F32, name="sig", tag="sig")
                    nc.vector.tensor_reduce(sig[:], parts[:, :nch],
                                            mybir.AxisListType.X,
                                            mybir.AluOpType.add)
                    nc.vector.reciprocal(recips[:, m:m + 1], sig[:])
                    for wi, (ex, c0, cw) in enumerate(exp_tiles):
                        wn = wnp.tile([P, 1024], F32, name="wn",
                                      tag="wn")[:, :cw]
                        weng = nc.vector if wi % 2 == 0 else nc.gpsimd
                        weng.tensor_scalar_mul(wn[:], ex.bitcast(F32),
                                               recips[:, m:m + 1])
                        nc.sync.dma_start(io["w_out"][m, :, c0:c0 + cw],
                                          wn[:])
                    if ext < L:
                        nc.sync.dma_start(io["w_out"][m, :, ext:L],
                                          zeros[:, :L - ext])
                    nc.vector.tensor_scalar_mul(attn[:, m, :], aps[:],
                                                recips[:, m:m + 1])
                    nc.vector.tensor_add(attn[:, m, :], attn[:, m, :],
                                         bvbc[:])
                    if mi == 5:
                        emit_gate_half(1)
                emit_gate_half(0)



# ======================================================================
# host side
# ======================================================================

def _prep_inputs(key, key_index, weights, biases):
    """Build the 8 per-core input dicts."""
    perm = np.concatenate([np.arange(0, E, 2), np.arange(1, E, 2)])
    inv = (1.0 / (np.float32(10000.0) **
                  (np.arange(0, E, 2).astype(np.float32) / np.float32(E))))
    inv = inv.astype(np.float32)

    stair = np.where(np.arange(P)[None, :] <= np.arange(P)[:, None],
                     np.float32(0.0), np.float32(NEG))
    mt = {
        0: np.concatenate([stair, np.full((P, P), NEG, np.float32)],
                          axis=1).astype(BF16_NP),
        1: np.concatenate([np.zeros((P, P), np.float32), stair],
                          axis=1).astype(BF16_NP),
    }

    wq, wk, wv, wo, wxr, wyr, wxz, wyz, wxg, wyg = weights
    bq, bk, bv, bo, bxr, byr, bxz, byz, bxg, byg = biases

    def lhsT(w):
        return np.ascontiguousarray(w.T).reshape(2, P, E)

    wts = np.stack([lhsT(wq[perm]), lhsT(wk[perm]), lhsT(wv), lhsT(wo),
                    lhsT(wxr), lhsT(wyr), lhsT(wxz), lhsT(wyz), lhsT(wxg),
                    lhsT(wyg)])                     # [10, 2, P, E]
    wts = np.ascontiguousarray(wts.transpose(2, 1, 0, 3))  # [P, 2, 10, E]
    biasv = np.ascontiguousarray(
        np.stack([bq[perm], bk[perm], -(bxr + byr), -(bxz + byz), bxg + byg, bo],
                 axis=1).reshape(2, P, 6).transpose(1, 0, 2)
        ).astype(np.float32)                        # [P, 2, 6]
    bvbc = np.broadcast_to(bv, (P, E)).astype(np.float32).copy()

    in_maps = []
    for b in range(B):
        keyT = np.ascontiguousarray(
            key[b].T.reshape(2, P, L).transpose(1, 0, 2))   # [P, 2, L]
        ang = key_index[b].astype(np.float32)[None, :] * inv[:, None]
        cosT = np.cos(ang).astype(BF16_NP)
        sinT = np.sin(ang).astype(BF16_NP)
        for h in range(2):
            qcols = (L - Q) + np.arange(Q).reshape(16, P)[h::2].reshape(-1)
            xT = np.ascontiguousarray(
                key[b][qcols].T.reshape(2, P, Q // 2).transpose(1, 0, 2))
            qcs = np.ascontiguousarray(
                np.stack([cosT[:, qcols],
                          sinT[:, qcols]]).transpose(1, 0, 2))
            in_maps.append(dict(
                keyT=keyT, cosT=cosT, sinT=sinT, mtail=np.asarray(mt[h]),
                biasv=biasv, bvbc=bvbc, xT_in=xT, qcs_in=qcs, wts=wts))
    return in_maps


def kernel(key, key_index, Wq, bq, Wk, bk, Wv, bv, Wo, bo,
           Wxr, bxr, Wyr, byr, Wxz, bxz, Wyz, byz, Wxg, bxg, Wyg, byg,
           query_length, **extra):
    key = np.asarray(key, np.float32)
    key_index = np.asarray(key_index)
    assert int(query_length) == Q and key.shape == (B, L, E)

    if "nc" not in _CACHE:
        _CACHE["nc"] = build_nc()
    nc = _CACHE["nc"]

    weights = [np.asarray(w, np.float32) for w in
               (Wq, Wk, Wv, Wo, Wxr, Wyr, Wxz, Wyz, Wxg, Wyg)]
    biases = [np.asarray(x, np.float32) for x in
              (bq, bk, bv, bo, bxr, byr, bxz, byz, bxg, byg)]
    in_maps = _prep_inputs(key, key_index, weights, biases)

    res = run_bass_kernel_spmd(nc, in_maps, core_ids=list(range(8)),
                               **_CACHE.get("run_kwargs", {}))
    out = np.empty((B, Q, E), np.float32)
    w = np.empty((B, Q, L), np.float32)
    for core in range(8):
        b, h = divmod(core, 2)
        r = res.results[core]
        out.reshape(B, 16, P, E)[b, h::2] = r["o_out"]
        w.reshape(B, 16, P, L)[b, h::2] = r["w_out"]
    _CACHE["last_result"] = res
    return out, w


# revision 28
# speedup vs baseline: 60918.0318x; 60918.0318x over previous
"""Bass/Trainium2 kernel for nn_EpisodeMultiheadAttentionBlock.

Reference computation (B=4, L=4096, E=256, Q=2048):
    x  = key[:, -Q:]
    q  = rope(x @ Wq.T + bq, idx[:, -Q:]);  k = rope(key @ Wk.T + bk, idx)
    v  = key @ Wv.T + bv
    s  = (q / sqrt(E)) @ k.T  with causal mask (bottom Q rows of triu)
    w  = softmax(s)                            -> output 2
    y  = (w @ v) @ Wo.T + bo
    r  = sigmoid(x@Wxr.T + y@Wyr.T); z = sigmoid(x@Wxz.T + y@Wyz.T)
    hh = tanh((r*x)@Wxg.T + y@Wyg.T)
    out = (1-z)*x + z*hh                       -> output 1

Sharding: 8 cores = 4 batches x 2 query-interleavings. Core (b, h) owns
query 128-row blocks g = 2m+h (m=0..7) of batch b. The interleaving makes
the causal key extent per local block m uniform across cores: E(m) = 18+2m
key blocks (even-parity cores carry one fully-masked pad block), so one
SPMD program serves all 8 cores; the parity-dependent boundary mask is a
tiny per-core input added into the score PSUM via an identity matmul.

Layouts: projections/gating run in "T layout" (feature dim on partitions)
so every matmul's operands arrive in the orientation the next matmul
consumes. RoPE's pair rotation becomes a between-subtile elementwise op by
permuting the feature dim of Wq/Wk to [evens | odds] host-side (the
permutation cancels inside q.k). Scores are computed naturally
([query-part, key-free]) so the softmax row-sum falls out of the scalar
engine's exp accum_out and w DMAs straight out; the attention matmul's
j-on-partitions operand is built with PE transposes of the exp'd tiles.
"""

import sys

sys.path.insert(0, "/opt/trn_rl_repo")

import numpy as np
import ml_dtypes

import concourse.bass as bass
import concourse.tile as tile
from concourse import bacc, mybir
from concourse.bass_utils import run_bass_kernel_spmd
from concourse.masks import make_identity

F32 = mybir.dt.float32
F32R = mybir.dt.float32r
BF16 = mybir.dt.bfloat16
BF16_NP = np.dtype(ml_dtypes.bfloat16)

B, L, E, Q = 4, 4096, 256, 2048
P = 128
NM = 8              # query 128-blocks per core
NEG = -30000.0      # additive mask; exp(NEG/16) underflows to exactly 0.0
SCALE = 1.0 / 16.0  # 1/sqrt(E)

IdF = mybir.ActivationFunctionType.Identity
ExpF = mybir.ActivationFunctionType.Exp
SigF = mybir.ActivationFunctionType.Sigmoid
TanhF = mybir.ActivationFunctionType.Tanh

_CACHE = {}


def _ext_blocks(m):
    """Causal key extent (in 128-col blocks) for local query block m."""
    return 18 + 2 * m


def build_nc():
    nc = bacc.Bacc("TRN2", target_bir_lowering=False, debug=False,
                   enable_asserts=False, num_devices=8)

    io = {}
    io["keyT"] = nc.dram_tensor("keyT", [P, 2, L], F32,
                                kind="ExternalInput").ap()
    io["xT_in"] = nc.dram_tensor("xT_in", [P, 2, Q // 2], F32,
                                 kind="ExternalInput").ap()
    io["cosT"] = nc.dram_tensor("cosT", [P, L], BF16,
                                kind="ExternalInput").ap()
    io["sinT"] = nc.dram_tensor("sinT", [P, L], BF16,
                                kind="ExternalInput").ap()
    io["qcs_in"] = nc.dram_tensor("qcs_in", [P, 2, Q // 2], BF16,
                                  kind="ExternalInput").ap()
    io["wts"] = nc.dram_tensor("wts", [P, 2, 10, E], F32,
                               kind="ExternalInput").ap()
    io["mtail"] = nc.dram_tensor("mtail", [P, 2 * P], BF16,
                                 kind="ExternalInput").ap()
    io["biasv"] = nc.dram_tensor("biasv", [P, 2, 6], F32,
                                 kind="ExternalInput").ap()
    io["bvbc"] = nc.dram_tensor("bvbc", [P, E], F32,
                                kind="ExternalInput").ap()
    io["w_out"] = nc.dram_tensor("w_out", [NM, P, L], F32,
                                 kind="ExternalOutput").ap()
    io["o_out"] = nc.dram_tensor("o_out", [NM, P, E], F32,
                                 kind="ExternalOutput").ap()

    with tile.TileContext(nc) as tc:
        _emit(nc, tc, io)
    nc.compile()
    return nc


def _emit(nc, tc, io):
    from contextlib import ExitStack

    with ExitStack() as ctx:
        const = ctx.enter_context(tc.tile_pool(name="const", bufs=1))
        big = ctx.enter_context(tc.tile_pool(name="big", bufs=1))
        # single PSUM pool layout (8 banks total):
        #   mmps [P,1024] x2bufs = 4 banks, tpps [P,512] x2 = 2,
        #   atps [P,256] x2 = 2
        mmps = ctx.enter_context(tc.tile_pool(name="mmps", bufs=2,
                                              space="PSUM"))
        tpps = ctx.enter_context(tc.tile_pool(name="tpps", bufs=2,
                                              space="PSUM"))
        atps = ctx.enter_context(tc.tile_pool(name="atps", bufs=2,
                                              space="PSUM"))

        def mm_tile(cols=1024):
            return mmps.tile([P, 1024], F32, name="mmt", tag="mm")[:, :cols]

        # ---- q path first: it gates the first score matmuls ----------
        wnames = ["wq", "wk", "wv", "wo", "wxr", "wyr", "wxz", "wyz",
                  "wxg", "wyg"]
        biasv = const.tile([P, 2, 6], F32)
        nc.sync.dma_start(biasv[:], io["biasv"])
        wts = const.tile([P, 2, 10, E], F32R)
        v = big.tile([P, L // P, E], F32R)         # 32KB/part
        xTr = big.tile([P, 2, Q // 2], F32R)       # 8KB (f32r-rounded x)

        with tc.tile_pool(name="rotp", bufs=1) as rotp:
            kTrot = rotp.tile([P, 2, L], BF16)     # 16KB
            qTrot = rotp.tile([P, 2, Q // 2], BF16)

            with tc.tile_pool(name="stgp", bufs=1) as stgp, \
                 tc.tile_pool(name="keyp", bufs=1) as keyp, \
                 tc.tile_pool(name="csp", bufs=1) as csp, \
                 tc.tile_pool(name="ropet", bufs=2) as ropet:
                stg = stgp.tile([P, 2, 10, E], F32, name="stg", tag="stg")
                xs = keyp.tile([P, 2, Q // 2], F32, name="xs", tag="ks",
                               bufs=2)
                nc.sync.dma_start(xs[:], io["xT_in"])
                nc.sync.dma_start(stg[:, :, :1, :], io["wts"][:, :, :1, :])
                nc.vector.tensor_copy(wts[:, :, :1, :], stg[:, :, :1, :])
                nc.vector.tensor_copy(xTr[:], xs[:])
                qcs = csp.tile([P, 2, Q // 2], BF16)
                nc.sync.dma_start(qcs[:], io["qcs_in"])
                nc.sync.dma_start(stg[:, :, 1:3, :], io["wts"][:, :, 1:3, :])
                nc.vector.tensor_copy(wts[:, :, 1:3, :], stg[:, :, 1:3, :])
                wt = {n: wts[:, :, i, :] for i, n in enumerate(wnames)}

                def project_rot(dst, src_rhs, wn_, cos_ap, sin_ap, bcol,
                                n, cb=None, CW=1024):
                    """dst[:,0,:] = a*c - b*s ; dst[:,1,:] = a*s + b*c
                    with (a,b) = halves of (W @ src + bias)."""
                    for ci, c0 in enumerate(range(0, n, CW)):
                        cw = min(CW, n - c0)
                        pa = mm_tile(cw)
                        pb = mm_tile(cw)
                        for dh, ps in ((0, pa), (1, pb)):
                            for s in range(2):
                                for q0 in range(0, cw, 512):
                                    qw = min(512, cw - q0)
                                    nc.tensor.matmul(
                                        ps[:, q0:q0 + qw],
                                        wt[wn_][:, s, dh * P:(dh + 1) * P],
                                        src_rhs[:, s, c0 + q0:c0 + q0 + qw],
                                        start=(s == 0), stop=(s == 1))
                        ab = ropet.tile([P, 2, 1024], BF16,
                                        name="ab", tag="ab")[:, :, :cw]
                        nc.scalar.activation(ab[:, 0, :], pa[:], IdF,
                                             bias=biasv[:, 0, bcol:bcol + 1])
                        nc.scalar.activation(ab[:, 1, :], pb[:], IdF,
                                             bias=biasv[:, 1, bcol:bcol + 1])
                        c_ap = cos_ap[:, c0:c0 + cw]
                        s_ap = sin_ap[:, c0:c0 + cw]
                        t1 = ropet.tile([P, 1024], BF16, name="t1",
                                        tag="t1", bufs=1)[:, :cw]
                        t2 = ropet.tile([P, 1024], BF16, name="t2",
                                        tag="t2", bufs=1)[:, :cw]
                        nc.vector.tensor_mul(t1[:], ab[:, 0, :], c_ap)
                        nc.vector.tensor_mul(t2[:], ab[:, 1, :], s_ap)
                        nc.vector.tensor_tensor(dst[:, 0, c0:c0 + cw],
                                                t1[:], t2[:],
                                                mybir.AluOpType.subtract)
                        t3 = ropet.tile([P, 1024], BF16, name="t3",
                                        tag="t3", bufs=2)[:, :cw]
                        t4 = ropet.tile([P, 1024], BF16, name="t4",
                                        tag="t4", bufs=2)[:, :cw]
                        nc.gpsimd.tensor_mul(t3[:], ab[:, 0, :], s_ap)
                        nc.vector.tensor_mul(t4[:], ab[:, 1, :], c_ap)
                        nc.vector.tensor_tensor(dst[:, 1, c0:c0 + cw],
                                                t3[:], t4[:],
                                                mybir.AluOpType.add)
                        if cb is not None:
                            cb(ci)

                # q projection + rope (only needs xs/wq/qcs -- starts fast)
                project_rot(qTrot, xTr, "wq", qcs[:, 0, :], qcs[:, 1, :],
                            0, Q // 2)

                # keyT load + round, chunk 0 first
                keyT = keyp.tile([P, 2, L], F32R)  # 32KB
                for ci, c0 in enumerate(range(0, L, 1024)):
                    ks = keyp.tile([P, 2, 1024], F32, name="ks", tag="ks",
                                   bufs=2)
                    nc.sync.dma_start(ks[:], io["keyT"][:, :, c0:c0 + 1024])
                    eng = nc.vector if ci % 2 == 0 else nc.gpsimd
                    eng.tensor_copy(keyT[:, :, c0:c0 + 1024], ks[:])
                cosT = csp.tile([P, L], BF16)
                sinT = csp.tile([P, L], BF16)
                nc.sync.dma_start(cosT[:], io["cosT"])
                nc.sync.dma_start(sinT[:], io["sinT"])

                # k projection + rope, with v-projection interleaved
                # (v group j4 covers keyT cols [512*j4, 512*j4+512) --
                #  exactly inside rope chunk ci = j4 // 2)
                def v_groups(ci):
                    for j4 in (2 * ci, 2 * ci + 1):
                        vps = mm_tile().rearrange("p (j d) -> p j d", d=E)
                        for jj in range(4):
                            J = j4 * 4 + jj
                            for s in range(2):
                                nc.tensor.matmul(
                                    vps[:, jj, :],
                                    keyT[:, s, J * P:(J + 1) * P],
                                    wt["wv"][:, s, :],
                                    start=(s == 0), stop=(s == 1))
                        dst = v[:, j4 * 4:(j4 + 1) * 4, :]
                        if j4 % 2 == 0:
                            nc.vector.tensor_copy(dst, vps[:])
                        else:
                            nc.scalar.copy(dst, vps[:])

                project_rot(kTrot, keyT, "wk", cosT, sinT, 1, L,
                            cb=v_groups)

                # remaining consts + gating weights (off critical path)
                ident = const.tile([P, P], F32)
                make_identity(nc, ident)
                identr = const.tile([P, P], F32R)
                nc.vector.tensor_copy(identr[:], ident[:])
                identb = const.tile([P, P], BF16)
                make_identity(nc, identb)
                mtail = const.tile([P, 2 * P], BF16)
                nc.sync.dma_start(mtail[:], io["mtail"])
                bvbc = const.tile([P, E], F32)
                nc.sync.dma_start(bvbc[:], io["bvbc"])
                nc.sync.dma_start(stg[:, :, 3:, :], io["wts"][:, :, 3:, :])
                nc.scalar.copy(wts[:, :, 3:, :], stg[:, :, 3:, :])

            # ---- main attention loop ---------------------------------
            with tc.tile_pool(name="expp", bufs=8) as expp, \
                 tc.tile_pool(name="wnp", bufs=4) as wnp, \
                 tc.tile_pool(name="wtp", bufs=6) as wtp, \
                 tc.tile_pool(name="zp", bufs=1) as zp, \
                 tc.tile_pool(name="gatep", bufs=1) as gatep, \
                 tc.tile_pool(name="smallp", bufs=2) as smallp:
                zeros = zp.tile([P, 1792], F32)
                nc.vector.memset(zeros[:], 0.0)
                attn = gatep.tile([P, NM, E], F32, name="attn")
                recips = gatep.tile([P, NM], F32, name="recips")

                def mm_Th(dst_sb, terms, bcol, act):
                    """dst[:,dh,:] = act(sum W.T-lhsT @ rhs[roff:] + b).
                    Sigmoid is computed as 0.5*tanh(t/2)+0.5 (biasv[bcol]
                    holds b/2) so the scalar engine never leaves the exp
                    table set -- avoids ~2.7us ACT table switches."""
                    for dh in range(2):
                        pr = atps.tile([P, 512], F32, name="pr",
                                       tag="aps")
                        ng = len(terms)
                        for gi, (wn_, rhs, roff) in enumerate(terms):
                            for s in range(2):
                                nc.tensor.matmul(
                                    pr[:],
                                    wt[wn_][:, s, dh * P:(dh + 1) * P],
                                    rhs[:, s, roff:roff + 512],
                                    start=(gi == 0 and s == 0),
                                    stop=(gi == ng - 1 and s == 1))
                        if act is SigF:
                            tmp = gatep.tile([P, 512], F32, name="sgt",
                                             tag="sgt", bufs=2)
                            nc.scalar.activation(tmp[:], pr[:], TanhF,
                                                 bias=biasv[:, dh,
                                                            bcol:bcol + 1],
                                                 scale=0.5)
                            nc.vector.tensor_scalar(dst_sb[:, dh, :],
                                                    tmp[:], 0.5, 0.5,
                                                    mybir.AluOpType.mult,
                                                    mybir.AluOpType.add)
                        else:
                            nc.scalar.activation(dst_sb[:, dh, :], pr[:],
                                                 act,
                                                 bias=biasv[:, dh,
                                                            bcol:bcol + 1])

                def emit_gate_half(half):
                    """GRU gating for query blocks [4*half, 4*half+4)."""
                    i0 = half * 512
                    attnT = gatep.tile([P, 2, 512], F32R, name="attnT",
                                       tag="attnT")
                    for dh in range(2):
                        tp = tpps.tile([P, 512], F32, name="tpg", tag="tp")
                        for mm in range(4):
                            m_ = half * 4 + mm
                            nc.tensor.transpose(
                                tp[:, mm * P:(mm + 1) * P],
                                attn[:, m_, dh * P:(dh + 1) * P],
                                ident[:])
                        if dh == 0:
                            nc.vector.tensor_copy(attnT[:, dh, :], tp[:])
                        else:
                            nc.scalar.copy(attnT[:, dh, :], tp[:])
                    yT = gatep.tile([P, 2, 512], F32R, name="yT", tag="yT")
                    mm_Th(yT, [("wo", attnT, 0)], 5, IdF)
                    rT = gatep.tile([P, 2, 512], F32, name="rT", tag="rT")
                    mm_Th(rT, [("wxr", xTr, i0), ("wyr", yT, 0)], 2, SigF)
                    zT = gatep.tile([P, 2, 512], F32, name="zT", tag="zT")
                    mm_Th(zT, [("wxz", xTr, i0), ("wyz", yT, 0)], 3, SigF)
                    xf = xTr.bitcast(F32)[:, :, i0:i0 + 512]
                    gx = gatep.tile([P, 2, 512], F32R, name="gx", tag="gx")
                    for dh in range(2):
                        nc.vector.tensor_mul(gx[:, dh, :], rT[:, dh, :],
                                             xf[:, dh, :])
                    hT = gatep.tile([P, 2, 512], F32, name="hT", tag="hT")
                    mm_Th(hT, [("wyg", yT, 0), ("wxg", gx, 0)], 4, TanhF)
                    outT = gatep.tile([P, 2, 512], F32, name="outT",
                                      tag="outT")
                    for dh in range(2):
                        d1 = gatep.tile([P, 512], F32, name="d1", tag="d1",
                                        bufs=2)
                        nc.vector.tensor_tensor(d1[:], hT[:, dh, :],
                                                xf[:, dh, :],
                                                mybir.AluOpType.subtract)
                        nc.vector.tensor_mul(d1[:], zT[:, dh, :], d1[:])
                        nc.vector.tensor_add(outT[:, dh, :], xf[:, dh, :],
                                             d1[:])
                    onat = gatep.tile([P, 4, E], F32, name="onat",
                                      tag="onat")
                    for dh in range(2):
                        tp = tpps.tile([P, 512], F32, name="tpo", tag="tp")
                        for mm in range(4):
                            nc.tensor.transpose(
                                tp[:, mm * P:(mm + 1) * P],
                                outT[:, dh, mm * P:(mm + 1) * P],
                                ident[:])
                        for mm in range(4):
                            dst = onat[:, mm, dh * P:(dh + 1) * P]
                            src = tp[:, mm * P:(mm + 1) * P]
                            if dh == 0:
                                nc.vector.tensor_copy(dst, src)
                            else:
                                nc.scalar.copy(dst, src)
                    nc.sync.dma_start(
                        io["o_out"].rearrange("m p e -> p m e")
                        [:, half * 4:half * 4 + 4, :], onat[:])

                for mi, m in enumerate([0, 1, 7, 6, 5, 4, 3, 2]):
                    EB = _ext_blocks(m)
                    ext = EB * P
                    nch = (ext + 1023) // 1024
                    parts = smallp.tile([P, 4], F32, name="parts",
                                        tag="parts")
                    aps = atps.tile([P, E], F32, name="aps", tag="aps")
                    exp_tiles = []
                    jdone = 0
                    for c in range(nch):
                        c0 = c * 1024
                        cw = min(1024, ext - c0)
                        sps = mm_tile(cw)
                        for q0 in range(0, cw, 512):
                            qw = min(512, cw - q0)
                            last = (c == nch - 1) and (q0 + qw == cw)
                            for s in range(2):
                                nc.tensor.matmul(
                                    sps[:, q0:q0 + qw],
                                    qTrot[:, s, m * P:(m + 1) * P],
                                    kTrot[:, s, c0 + q0:c0 + q0 + qw],
                                    start=(s == 0),
                                    stop=(s == 1) and not last)
                            if last:
                                nc.tensor.matmul(
                                    sps[:, cw - 2 * P:cw], identb[:],
                                    mtail[:], start=False, stop=True,
                                    skip_group_check=True)
                        ex = expp.tile([P, 1024], F32R, name="ex",
                                       tag="ex")[:, :cw]
                        nc.scalar.activation(ex[:], sps[:], ExpF, bias=0.0,
                                             scale=SCALE,
                                             accum_out=parts[:, c:c + 1])
                        exp_tiles.append((ex, c0, cw))
                        for g0 in range(0, cw, 512):
                            gw = min(512, cw - g0)
                            nblk = gw // P
                            tp = tpps.tile([P, 512], F32R, name="tp",
                                           tag="tp")[:, :gw]
                            for jj in range(nblk):
                                nc.tensor.transpose(
                                    tp[:, jj * P:(jj + 1) * P],
                                    ex[:, g0 + jj * P:g0 + (jj + 1) * P],
                                    identr[:])
                            wTs = wtp.tile([P, 512], F32R, name="wTs",
                                           tag="wTs")[:, :gw]
                            if (jdone // 4) % 2 == 0:
                                nc.vector.tensor_copy(wTs[:], tp[:])
                            else:
                                nc.scalar.copy(wTs[:], tp[:])
                            for jj in range(nblk):
                                J = jdone + jj
                                nc.tensor.matmul(
                                    aps[:],
                                    wTs[:, jj * P:(jj + 1) * P],
                                    v[:, J, :],
                                    start=(J == 0), stop=(J == EB - 1))
                            jdone += nblk
                    sig = smallp.tile([P, 1], F32, name="sig", tag="sig")
                    nc.vector.tensor_reduce(sig[:], parts[:, :nch],
                                            mybir.AxisListType.X,
                                            mybir.AluOpType.add)
                    nc.vector.reciprocal(recips[:, m:m + 1], sig[:])
                    for wi, (ex, c0, cw) in enumerate(exp_tiles):
                        wn = wnp.tile([P, 1024], F32, name="wn",
                                      tag="wn")[:, :cw]
                        weng = nc.vector if wi % 2 == 0 else nc.gpsimd
                        weng.tensor_scalar_mul(wn[:], ex.bitcast(F32),
                                               recips[:, m:m + 1])
                        nc.sync.dma_start(io["w_out"][m, :, c0:c0 + cw],
                                          wn[:])
                    if ext < L:
                        nc.sync.dma_start(io["w_out"][m, :, ext:L],
                                          zeros[:, :L - ext])
                    nc.vector.tensor_scalar_mul(attn[:, m, :], aps[:],
                                                recips[:, m:m + 1])
                    nc.vector.tensor_add(attn[:, m, :], attn[:, m, :],
                                         bvbc[:])
                    if mi == 5:
                        emit_gate_half(1)
                emit_gate_half(0)



# ======================================================================
# host side
# ======================================================================

def _prep_inputs(key, key_index, weights, biases):
    """Build the 8 per-core input dicts."""
    perm = np.concatenate([np.arange(0, E, 2), np.arange(1, E, 2)])
    inv = (1.0 / (np.float32(10000.0) **
                  (np.arange(0, E, 2).astype(np.float32) / np.float32(E))))
    inv = inv.astype(np.float32)

    stair = np.where(np.arange(P)[None, :] <= np.arange(P)[:, None],
                     np.float32(0.0), np.float32(NEG))
    mt = {
        0: np.concatenate([stair, np.full((P, P), NEG, np.float32)],
                          axis=1).astype(BF16_NP),
        1: np.concatenate([np.zeros((P, P), np.float32), stair],
                          axis=1).astype(BF16_NP),
    }

    wq, wk, wv, wo, wxr, wyr, wxz, wyz, wxg, wyg = weights
    bq, bk, bv, bo, bxr, byr, bxz, byz, bxg, byg = biases

    def lhsT(w):
        return np.ascontiguousarray(w.T).reshape(2, P, E)

    wts = np.stack([lhsT(wq[perm]), lhsT(wk[perm]), lhsT(wv), lhsT(wo),
                    lhsT(wxr), lhsT(wyr), lhsT(wxz), lhsT(wyz), lhsT(wxg),
                    lhsT(wyg)])                     # [10, 2, P, E]
    wts = np.ascontiguousarray(wts.transpose(2, 1, 0, 3))  # [P, 2, 10, E]
    biasv = np.ascontiguousarray(
        np.stack([bq[perm], bk[perm], 0.5 * (bxr + byr), 0.5 * (bxz + byz), bxg + byg, bo],
                 axis=1).reshape(2, P, 6).transpose(1, 0, 2)
        ).astype(np.float32)                        # [P, 2, 6]
    bvbc = np.broadcast_to(bv, (P, E)).astype(np.float32).copy()

    in_maps = []
    for b in range(B):
        keyT = np.ascontiguousarray(
            key[b].T.reshape(2, P, L).transpose(1, 0, 2))   # [P, 2, L]
        ang = key_index[b].astype(np.float32)[None, :] * inv[:, None]
        cosT = np.cos(ang).astype(BF16_NP)
        sinT = np.sin(ang).astype(BF16_NP)
        for h in range(2):
            qcols = (L - Q) + np.arange(Q).reshape(16, P)[h::2].reshape(-1)
            xT = np.ascontiguousarray(
                key[b][qcols].T.reshape(2, P, Q // 2).transpose(1, 0, 2))
            qcs = np.ascontiguousarray(
                np.stack([cosT[:, qcols],
                          sinT[:, qcols]]).transpose(1, 0, 2))
            in_maps.append(dict(
                keyT=keyT, cosT=cosT, sinT=sinT, mtail=np.asarray(mt[h]),
                biasv=biasv, bvbc=bvbc, xT_in=xT, qcs_in=qcs, wts=wts))
    return in_maps


def kernel(key, key_index, Wq, bq, Wk, bk, Wv, bv, Wo, bo,
           Wxr, bxr, Wyr, byr, Wxz, bxz, Wyz, byz, Wxg, bxg, Wyg, byg,
           query_length, **extra):
    key = np.asarray(key, np.float32)
    key_index = np.asarray(key_index)
    assert int(query_length) == Q and key.shape == (B, L, E)

    if "nc" not in _CACHE:
        _CACHE["nc"] = build_nc()
    nc = _CACHE["nc"]

    weights = [np.asarray(w, np.float32) for w in
               (Wq, Wk, Wv, Wo, Wxr, Wyr, Wxz, Wyz, Wxg, Wyg)]
    biases = [np.asarray(x, np.float32) for x in
              (bq, bk, bv, bo, bxr, byr, bxz, byz, bxg, byg)]
    in_maps = _prep_inputs(key, key_index, weights, biases)

    res = run_bass_kernel_spmd(nc, in_maps, core_ids=list(range(8)),
                               **_CACHE.get("run_kwargs", {}))
    out = np.empty((B, Q, E), np.float32)
    w = np.empty((B, Q, L), np.float32)
    for core in range(8):
        b, h = divmod(core, 2)
        r = res.results[core]
        out.reshape(B, 16, P, E)[b, h::2] = r["o_out"]
        w.reshape(B, 16, P, L)[b, h::2] = r["w_out"]
    _CACHE["last_result"] = res
    return out, w


# revision 40
# speedup vs baseline: 61897.5968x; 1.0161x over previous
"""Bass/Trainium2 kernel for nn_EpisodeMultiheadAttentionBlock.

Reference computation (B=4, L=4096, E=256, Q=2048):
    x  = key[:, -Q:]
    q  = rope(x @ Wq.T + bq, idx[:, -Q:]);  k = rope(key @ Wk.T + bk, idx)
    v  = key @ Wv.T + bv
    s  = (q / sqrt(E)) @ k.T  with causal mask (bottom Q rows of triu)
    w  = softmax(s)                            -> output 2
    y  = (w @ v) @ Wo.T + bo
    r  = sigmoid(x@Wxr.T + y@Wyr.T); z = sigmoid(x@Wxz.T + y@Wyz.T)
    hh = tanh((r*x)@Wxg.T + y@Wyg.T)
    out = (1-z)*x + z*hh                       -> output 1

Sharding: 8 cores = 4 batches x 2 query-interleavings. Core (b, h) owns
query 128-row blocks g = 2m+h (m=0..7) of batch b. The interleaving makes
the causal key extent per local block m uniform across cores: E(m) = 18+2m
key blocks (even-parity cores carry one fully-masked pad block), so one
SPMD program serves all 8 cores; the parity-dependent boundary mask is a
tiny per-core input added into the score PSUM via an identity matmul.

Layouts: projections/gating run in "T layout" (feature dim on partitions)
so every matmul's operands arrive in the orientation the next matmul
consumes. RoPE's pair rotation becomes a between-subtile elementwise op by
permuting the feature dim of Wq/Wk to [evens | odds] host-side (the
permutation cancels inside q.k). Scores are computed naturally
([query-part, key-free]) so the softmax row-sum falls out of the scalar
engine's exp accum_out and w DMAs straight out; the attention matmul's
j-on-partitions operand is built with PE transposes of the exp'd tiles.
"""

import sys

sys.path.insert(0, "/opt/trn_rl_repo")

import numpy as np
import ml_dtypes

import concourse.bass as bass
import concourse.tile as tile
from concourse import bacc, mybir
from concourse.bass_utils import run_bass_kernel_spmd
from concourse.masks import make_identity

F32 = mybir.dt.float32
F32R = mybir.dt.float32r
BF16 = mybir.dt.bfloat16
BF16_NP = np.dtype(ml_dtypes.bfloat16)

B, L, E, Q = 4, 4096, 256, 2048
P = 128
NM = 8              # query 128-blocks per core
NEG = -30000.0      # additive mask; exp(NEG/16) underflows to exactly 0.0
SCALE = 1.0 / 16.0  # 1/sqrt(E)

IdF = mybir.ActivationFunctionType.Identity
ExpF = mybir.ActivationFunctionType.Exp
SigF = mybir.ActivationFunctionType.Sigmoid
TanhF = mybir.ActivationFunctionType.Tanh

_CACHE = {}


def _ext_blocks(m):
    """Causal key extent (in 128-col blocks) for local query block m."""
    return 18 + 2 * m


def build_nc():
    nc = bacc.Bacc("TRN2", target_bir_lowering=False, debug=False,
                   enable_asserts=False, num_devices=8)

    io = {}
    io["keyT"] = nc.dram_tensor("keyT", [P, 2, L], F32,
                                kind="ExternalInput").ap()
    io["xT_in"] = nc.dram_tensor("xT_in", [P, 2, Q // 2], F32,
                                 kind="ExternalInput").ap()
    io["cosT"] = nc.dram_tensor("cosT", [P, L], BF16,
                                kind="ExternalInput").ap()
    io["sinT"] = nc.dram_tensor("sinT", [P, L], BF16,
                                kind="ExternalInput").ap()
    io["qcs_in"] = nc.dram_tensor("qcs_in", [P, 2, Q // 2], BF16,
                                  kind="ExternalInput").ap()
    io["wts"] = nc.dram_tensor("wts", [P, 2, 10, E], F32,
                               kind="ExternalInput").ap()
    io["mtail"] = nc.dram_tensor("mtail", [P, 2 * P], BF16,
                                 kind="ExternalInput").ap()
    io["biasv"] = nc.dram_tensor("biasv", [P, 2, 6], F32,
                                 kind="ExternalInput").ap()
    io["bvbc"] = nc.dram_tensor("bvbc", [P, E], F32,
                                kind="ExternalInput").ap()
    io["w_out"] = nc.dram_tensor("w_out", [NM, P, L], F32,
                                 kind="ExternalOutput").ap()
    io["o_out"] = nc.dram_tensor("o_out", [NM, P, E], F32,
                                 kind="ExternalOutput").ap()

    with tile.TileContext(nc) as tc:
        _emit(nc, tc, io)
    nc.compile()
    return nc


def _emit(nc, tc, io):
    from contextlib import ExitStack

    with ExitStack() as ctx:
        const = ctx.enter_context(tc.tile_pool(name="const", bufs=1))
        big = ctx.enter_context(tc.tile_pool(name="big", bufs=1))
        # single PSUM pool layout (8 banks total):
        #   mmps [P,1024] x2bufs = 4 banks, tpps [P,512] x2 = 2,
        #   atps [P,256] x2 = 2
        mmps = ctx.enter_context(tc.tile_pool(name="mmps", bufs=2,
                                              space="PSUM"))
        tpps = ctx.enter_context(tc.tile_pool(name="tpps", bufs=2,
                                              space="PSUM"))
        atps = ctx.enter_context(tc.tile_pool(name="atps", bufs=2,
                                              space="PSUM"))

        def mm_tile(cols=1024):
            return mmps.tile([P, 1024], F32, name="mmt", tag="mm")[:, :cols]

        # ---- q path first: it gates the first score matmuls ----------
        wnames = ["wq", "wk", "wv", "wo", "wxr", "wyr", "wxz", "wyz",
                  "wxg", "wyg"]
        biasv = const.tile([P, 2, 6], F32)
        nc.sync.dma_start(biasv[:], io["biasv"])
        wts = const.tile([P, 2, 10, E], F32R)
        v = big.tile([P, L // P, E], F32R)         # 32KB/part
        xTr = big.tile([P, 2, Q // 2], F32R)       # 8KB (f32r-rounded x)

        with tc.tile_pool(name="rotp", bufs=1) as rotp:
            kTrot = rotp.tile([P, 2, L], BF16)     # 16KB
            qTrot = rotp.tile([P, 2, Q // 2], BF16)

            with tc.tile_pool(name="stgp", bufs=1) as stgp, \
                 tc.tile_pool(name="keyp", bufs=1) as keyp, \
                 tc.tile_pool(name="csp", bufs=1) as csp, \
                 tc.tile_pool(name="ropet", bufs=2) as ropet:
                stg = stgp.tile([P, 2, 10, E], F32, name="stg", tag="stg")
                xs = keyp.tile([P, 2, Q // 2], F32, name="xs", tag="ks",
                               bufs=2)
                nc.sync.dma_start(xs[:], io["xT_in"])
                nc.sync.dma_start(stg[:, :, :1, :], io["wts"][:, :, :1, :])
                nc.vector.tensor_copy(wts[:, :, :1, :], stg[:, :, :1, :])
                nc.vector.tensor_copy(xTr[:], xs[:])
                qcs = csp.tile([P, 2, Q // 2], BF16)
                nc.sync.dma_start(qcs[:], io["qcs_in"])
                nc.sync.dma_start(stg[:, :, 1:3, :], io["wts"][:, :, 1:3, :])
                nc.vector.tensor_copy(wts[:, :, 1:3, :], stg[:, :, 1:3, :])
                wt = {n: wts[:, :, i, :] for i, n in enumerate(wnames)}

                def project_rot(dst, src_rhs, wn_, cos_ap, sin_ap, bcol,
                                n, cb=None, CW=1024):
                    """dst[:,0,:] = a*c - b*s ; dst[:,1,:] = a*s + b*c
                    with (a,b) = halves of (W @ src + bias)."""
                    for ci, c0 in enumerate(range(0, n, CW)):
                        cw = min(CW, n - c0)
                        pa = mm_tile(cw)
                        pb = mm_tile(cw)
                        for dh, ps in ((0, pa), (1, pb)):
                            for s in range(2):
                                for q0 in range(0, cw, 512):
                                    qw = min(512, cw - q0)
                                    nc.tensor.matmul(
                                        ps[:, q0:q0 + qw],
                                        wt[wn_][:, s, dh * P:(dh + 1) * P],
                                        src_rhs[:, s, c0 + q0:c0 + q0 + qw],
                                        start=(s == 0), stop=(s == 1))
                        ab = ropet.tile([P, 2, 1024], BF16,
                                        name="ab", tag="ab")[:, :, :cw]
                        nc.scalar.activation(ab[:, 0, :], pa[:], IdF,
                                             bias=biasv[:, 0, bcol:bcol + 1])
                        nc.scalar.activation(ab[:, 1, :], pb[:], IdF,
                                             bias=biasv[:, 1, bcol:bcol + 1])
                        c_ap = cos_ap[:, c0:c0 + cw]
                        s_ap = sin_ap[:, c0:c0 + cw]
                        t1 = ropet.tile([P, 1024], BF16, name="t1",
                                        tag="t1", bufs=1)[:, :cw]
                        t2 = ropet.tile([P, 1024], BF16, name="t2",
                                        tag="t2", bufs=1)[:, :cw]
                        nc.vector.tensor_mul(t1[:], ab[:, 0, :], c_ap)
                        nc.vector.tensor_mul(t2[:], ab[:, 1, :], s_ap)
                        nc.vector.tensor_tensor(dst[:, 0, c0:c0 + cw],
                                                t1[:], t2[:],
                                                mybir.AluOpType.subtract)
                        t3 = ropet.tile([P, 1024], BF16, name="t3",
                                        tag="t3", bufs=2)[:, :cw]
                        t4 = ropet.tile([P, 1024], BF16, name="t4",
                                        tag="t4", bufs=2)[:, :cw]
                        nc.gpsimd.tensor_mul(t3[:], ab[:, 0, :], s_ap)
                        nc.vector.tensor_mul(t4[:], ab[:, 1, :], c_ap)
                        nc.vector.tensor_tensor(dst[:, 1, c0:c0 + cw],
                                                t3[:], t4[:],
                                                mybir.AluOpType.add)
                        if cb is not None:
                            cb(ci)

                # q projection + rope (only needs xs/wq/qcs -- starts fast)
                project_rot(qTrot, xTr, "wq", qcs[:, 0, :], qcs[:, 1, :],
                            0, Q // 2)

                # keyT load + round, chunk 0 first
                keyT = keyp.tile([P, 2, L], F32R)  # 32KB
                for ci, c0 in enumerate(range(0, L, 1024)):
                    ks = keyp.tile([P, 2, 1024], F32, name="ks", tag="ks",
                                   bufs=2)
                    nc.sync.dma_start(ks[:], io["keyT"][:, :, c0:c0 + 1024])
                    nc.gpsimd.tensor_copy(keyT[:, :, c0:c0 + 1024], ks[:])
                cosT = csp.tile([P, L], BF16)
                sinT = csp.tile([P, L], BF16)
                nc.sync.dma_start(cosT[:], io["cosT"])
                nc.sync.dma_start(sinT[:], io["sinT"])

                # k projection + rope, with v-projection interleaved
                # (v group j4 covers keyT cols [512*j4, 512*j4+512) --
                #  exactly inside rope chunk ci = j4 // 2)
                def v_groups(ci):
                    for j4 in (2 * ci, 2 * ci + 1):
                        vps = mm_tile().rearrange("p (j d) -> p j d", d=E)
                        for jj in range(4):
                            J = j4 * 4 + jj
                            for s in range(2):
                                nc.tensor.matmul(
                                    vps[:, jj, :],
                                    keyT[:, s, J * P:(J + 1) * P],
                                    wt["wv"][:, s, :],
                                    start=(s == 0), stop=(s == 1))
                        dst = v[:, j4 * 4:(j4 + 1) * 4, :]
                        if j4 % 2 == 0:
                            nc.vector.tensor_copy(dst, vps[:])
                        else:
                            nc.scalar.copy(dst, vps[:])

                project_rot(kTrot, keyT, "wk", cosT, sinT, 1, L,
                            cb=v_groups)

                # remaining consts + gating weights (off critical path)
                ident = const.tile([P, P], F32)
                make_identity(nc, ident)
                identr = const.tile([P, P], F32R)
                nc.vector.tensor_copy(identr[:], ident[:])
                identb = const.tile([P, P], BF16)
                make_identity(nc, identb)
                mtail = const.tile([P, 2 * P], BF16)
                nc.sync.dma_start(mtail[:], io["mtail"])
                bvbc = const.tile([P, E], F32)
                nc.sync.dma_start(bvbc[:], io["bvbc"])
                nc.sync.dma_start(stg[:, :, 3:, :], io["wts"][:, :, 3:, :])
                nc.scalar.copy(wts[:, :, 3:, :], stg[:, :, 3:, :])

            # ---- main attention loop ---------------------------------
            with tc.tile_pool(name="expp", bufs=8) as expp, \
                 tc.tile_pool(name="wnp", bufs=4) as wnp, \
                 tc.tile_pool(name="wtp", bufs=6) as wtp, \
                 tc.tile_pool(name="zp", bufs=1) as zp, \
                 tc.tile_pool(name="gatep", bufs=1) as gatep, \
                 tc.tile_pool(name="smallp", bufs=2) as smallp:
                zeros = zp.tile([P, 1792], F32)
                nc.vector.memset(zeros[:], 0.0)
                attn = gatep.tile([P, NM, E], F32, name="attn")
                recips = gatep.tile([P, NM], F32, name="recips")

                def mm_Th(dst_sb, terms, bcol, act):
                    """dst[:,dh,:] = act(sum W.T-lhsT @ rhs[roff:] + b).
                    Sigmoid is computed as 0.5*tanh(t/2)+0.5 (biasv[bcol]
                    holds b/2) so the scalar engine never leaves the exp
                    table set -- avoids ~2.7us ACT table switches."""
                    for dh in range(2):
                        pr = atps.tile([P, 512], F32, name="pr",
                                       tag="aps")
                        ng = len(terms)
                        for gi, (wn_, rhs, roff) in enumerate(terms):
                            for s in range(2):
                                nc.tensor.matmul(
                                    pr[:],
                                    wt[wn_][:, s, dh * P:(dh + 1) * P],
                                    rhs[:, s, roff:roff + 512],
                                    start=(gi == 0 and s == 0),
                                    stop=(gi == ng - 1 and s == 1))
                        if act is SigF:
                            tmp = gatep.tile([P, 512], F32, name="sgt",
                                             tag="sgt", bufs=2)
                            nc.scalar.activation(tmp[:], pr[:], TanhF,
                                                 bias=biasv[:, dh,
                                                            bcol:bcol + 1],
                                                 scale=0.5)
                            nc.vector.tensor_scalar(dst_sb[:, dh, :],
                                                    tmp[:], 0.5, 0.5,
                                                    mybir.AluOpType.mult,
                                                    mybir.AluOpType.add)
                        else:
                            nc.scalar.activation(dst_sb[:, dh, :], pr[:],
                                                 act,
                                                 bias=biasv[:, dh,
                                                            bcol:bcol + 1])

                def emit_gate_half(half):
                    """GRU gating for query blocks [4*half, 4*half+4)."""
                    i0 = half * 512
                    attnT = gatep.tile([P, 2, 512], F32R, name="attnT",
                                       tag="attnT")
                    for dh in range(2):
                        tp = tpps.tile([P, 512], F32, name="tpg", tag="tp")
                        for mm in range(4):
                            m_ = half * 4 + mm
                            nc.tensor.transpose(
                                tp[:, mm * P:(mm + 1) * P],
                                attn[:, m_, dh * P:(dh + 1) * P],
                                ident[:])
                        if dh == 0:
                            nc.vector.tensor_copy(attnT[:, dh, :], tp[:])
                        else:
                            nc.scalar.copy(attnT[:, dh, :], tp[:])
                    yT = gatep.tile([P, 2, 512], F32R, name="yT", tag="yT")
                    mm_Th(yT, [("wo", attnT, 0)], 5, IdF)
                    rT = gatep.tile([P, 2, 512], F32, name="rT", tag="rT")
                    mm_Th(rT, [("wxr", xTr, i0), ("wyr", yT, 0)], 2, SigF)
                    zT = gatep.tile([P, 2, 512], F32, name="zT", tag="zT")
                    mm_Th(zT, [("wxz", xTr, i0), ("wyz", yT, 0)], 3, SigF)
                    xf = xTr.bitcast(F32)[:, :, i0:i0 + 512]
                    gx = gatep.tile([P, 2, 512], F32R, name="gx", tag="gx")
                    for dh in range(2):
                        nc.vector.tensor_mul(gx[:, dh, :], rT[:, dh, :],
                                             xf[:, dh, :])
                    hT = gatep.tile([P, 2, 512], F32, name="hT", tag="hT")
                    mm_Th(hT, [("wyg", yT, 0), ("wxg", gx, 0)], 4, TanhF)
                    outT = gatep.tile([P, 2, 512], F32, name="outT",
                                      tag="outT")
                    for dh in range(2):
                        d1 = gatep.tile([P, 512], F32, name="d1", tag="d1",
                                        bufs=2)
                        nc.vector.tensor_tensor(d1[:], hT[:, dh, :],
                                                xf[:, dh, :],
                                                mybir.AluOpType.subtract)
                        nc.vector.tensor_mul(d1[:], zT[:, dh, :], d1[:])
                        nc.vector.tensor_add(outT[:, dh, :], xf[:, dh, :],
                                             d1[:])
                    onat = gatep.tile([P, 4, E], F32, name="onat",
                                      tag="onat")
                    for dh in range(2):
                        tp = tpps.tile([P, 512], F32, name="tpo", tag="tp")
                        for mm in range(4):
                            nc.tensor.transpose(
                                tp[:, mm * P:(mm + 1) * P],
                                outT[:, dh, mm * P:(mm + 1) * P],
                                ident[:])
                        for mm in range(4):
                            dst = onat[:, mm, dh * P:(dh + 1) * P]
                            src = tp[:, mm * P:(mm + 1) * P]
                            if dh == 0:
                                nc.vector.tensor_copy(dst, src)
                            else:
                                nc.scalar.copy(dst, src)
                    nc.sync.dma_start(
                        io["o_out"].rearrange("m p e -> p m e")
                        [:, half * 4:half * 4 + 4, :], onat[:])

                for mi, m in enumerate([0, 1, 7, 6, 5, 4, 3, 2]):
                    EB = _ext_blocks(m)
                    ext = EB * P
                    nch = (ext + 1023) // 1024
                    parts = smallp.tile([P, 4], F32, name="parts",
                                        tag="parts")
                    aps = atps.tile([P, E], F32, name="aps", tag="aps")
                    exp_tiles = []
                    jdone = 0
                    for c in range(nch):
                        c0 = c * 1024
                        cw = min(1024, ext - c0)
                        sps = mm_tile(cw)
                        for q0 in range(0, cw, 512):
                            qw = min(512, cw - q0)
                            last = (c == nch - 1) and (q0 + qw == cw)
                            for s in range(2):
                                nc.tensor.matmul(
                                    sps[:, q0:q0 + qw],
                                    qTrot[:, s, m * P:(m + 1) * P],
                                    kTrot[:, s, c0 + q0:c0 + q0 + qw],
                                    start=(s == 0),
                                    stop=(s == 1) and not last)
                            if last:
                                nc.tensor.matmul(
                                    sps[:, cw - 2 * P:cw], identb[:],
                                    mtail[:], start=False, stop=True,
                                    skip_group_check=True)
                        ex = expp.tile([P, 1024], F32R, name="ex",
                                       tag="ex")[:, :cw]
                        nc.scalar.activation(ex[:], sps[:], ExpF, bias=0.0,
                                             scale=SCALE,
                                             accum_out=parts[:, c:c + 1])
                        exp_tiles.append((ex, c0, cw))
                        for g0 in range(0, cw, 512):
                            gw = min(512, cw - g0)
                            nblk = gw // P
                            tp = tpps.tile([P, 512], F32R, name="tp",
                                           tag="tp")[:, :gw]
                            for jj in range(nblk):
                                nc.tensor.transpose(
                                    tp[:, jj * P:(jj + 1) * P],
                                    ex[:, g0 + jj * P:g0 + (jj + 1) * P],
                                    identr[:])
                            wTs = wtp.tile([P, 512], F32R, name="wTs",
                                           tag="wTs")[:, :gw]
                            if (jdone // 4) % 2 == 0:
                                nc.vector.tensor_copy(wTs[:], tp[:])
                            else:
                                nc.scalar.copy(wTs[:], tp[:])
                            for jj in range(nblk):
                                J = jdone + jj
                                nc.tensor.matmul(
                                    aps[:],
                                    wTs[:, jj * P:(jj + 1) * P],
                                    v[:, J, :],
                                    start=(J == 0), stop=(J == EB - 1))
                            jdone += nblk
                    sig = smallp.tile([P, 1], F32, name="sig", tag="sig")
                    nc.vector.tensor_reduce(sig[:], parts[:, :nch],
                                            mybir.AxisListType.X,
                                            mybir.AluOpType.add)
                    nc.vector.reciprocal(recips[:, m:m + 1], sig[:])
                    for wi, (ex, c0, cw) in enumerate(exp_tiles):
                        wn = wnp.tile([P, 1024], F32, name="wn",
                                      tag="wn")[:, :cw]
                        weng = nc.vector if wi % 2 == 0 else nc.gpsimd
                        weng.tensor_scalar_mul(wn[:], ex.bitcast(F32),
                                               recips[:, m:m + 1])
                        nc.sync.dma_start(io["w_out"][m, :, c0:c0 + cw],
                                          wn[:])
                    if ext < L:
                        nc.sync.dma_start(io["w_out"][m, :, ext:L],
                                          zeros[:, :L - ext])
                    nc.vector.tensor_scalar_mul(attn[:, m, :], aps[:],
                                                recips[:, m:m + 1])
                    nc.vector.tensor_add(attn[:, m, :], attn[:, m, :],
                                         bvbc[:])
                    if mi == 5:
                        emit_gate_half(1)
                emit_gate_half(0)



# ======================================================================
# host side
# ======================================================================

def _prep_inputs(key, key_index, weights, biases):
    """Build the 8 per-core input dicts."""
    perm = np.concatenate([np.arange(0, E, 2), np.arange(1, E, 2)])
    inv = (1.0 / (np.float32(10000.0) **
                  (np.arange(0, E, 2).astype(np.float32) / np.float32(E))))
    inv = inv.astype(np.float32)

    stair = np.where(np.arange(P)[None, :] <= np.arange(P)[:, None],
                     np.float32(0.0), np.float32(NEG))
    mt = {
        0: np.concatenate([stair, np.full((P, P), NEG, np.float32)],
                          axis=1).astype(BF16_NP),
        1: np.concatenate([np.zeros((P, P), np.float32), stair],
                          axis=1).astype(BF16_NP),
    }

    wq, wk, wv, wo, wxr, wyr, wxz, wyz, wxg, wyg = weights
    bq, bk, bv, bo, bxr, byr, bxz, byz, bxg, byg = biases

    def lhsT(w):
        return np.ascontiguousarray(w.T).reshape(2, P, E)

    wts = np.stack([lhsT(wq[perm]), lhsT(wk[perm]), lhsT(wv), lhsT(wo),
                    lhsT(wxr), lhsT(wyr), lhsT(wxz), lhsT(wyz), lhsT(wxg),
                    lhsT(wyg)])                     # [10, 2, P, E]
    wts = np.ascontiguousarray(wts.transpose(2, 1, 0, 3))  # [P, 2, 10, E]
    biasv = np.ascontiguousarray(
        np.stack([bq[perm], bk[perm], 0.5 * (bxr + byr), 0.5 * (bxz + byz), bxg + byg, bo],
                 axis=1).reshape(2, P, 6).transpose(1, 0, 2)
        ).astype(np.float32)                        # [P, 2, 6]
    bvbc = np.broadcast_to(bv, (P, E)).astype(np.float32).copy()

    in_maps = []
    for b in range(B):
        keyT = np.ascontiguousarray(
            key[b].T.reshape(2, P, L).transpose(1, 0, 2))   # [P, 2, L]
        ang = key_index[b].astype(np.float32)[None, :] * inv[:, None]
        cosT = np.cos(ang).astype(BF16_NP)
        sinT = np.sin(ang).astype(BF16_NP)
        for h in range(2):
            qcols = (L - Q) + np.arange(Q).reshape(16, P)[h::2].reshape(-1)
            xT = np.ascontiguousarray(
                key[b][qcols].T.reshape(2, P, Q // 2).transpose(1, 0, 2))
            qcs = np.ascontiguousarray(
                np.stack([cosT[:, qcols],
                          sinT[:, qcols]]).transpose(1, 0, 2))
            in_maps.append(dict(
                keyT=keyT, cosT=cosT, sinT=sinT, mtail=np.asarray(mt[h]),
                biasv=biasv, bvbc=bvbc, xT_in=xT, qcs_in=qcs, wts=wts))
    return in_maps


def kernel(key, key_index, Wq, bq, Wk, bk, Wv, bv, Wo, bo,
           Wxr, bxr, Wyr, byr, Wxz, bxz, Wyz, byz, Wxg, bxg, Wyg, byg,
           query_length, **extra):
    key = np.asarray(key, np.float32)
    key_index = np.asarray(key_index)
    assert int(query_length) == Q and key.shape == (B, L, E)

    if "nc" not in _CACHE:
        _CACHE["nc"] = build_nc()
    nc = _CACHE["nc"]

    weights = [np.asarray(w, np.float32) for w in
               (Wq, Wk, Wv, Wo, Wxr, Wyr, Wxz, Wyz, Wxg, Wyg)]
    biases = [np.asarray(x, np.float32) for x in
              (bq, bk, bv, bo, bxr, byr, bxz, byz, bxg, byg)]
    in_maps = _prep_inputs(key, key_index, weights, biases)

    res = run_bass_kernel_spmd(nc, in_maps, core_ids=list(range(8)),
                               **_CACHE.get("run_kwargs", {}))
    out = np.empty((B, Q, E), np.float32)
    w = np.empty((B, Q, L), np.float32)
    for core in range(8):
        b, h = divmod(core, 2)
        r = res.results[core]
        out.reshape(B, 16, P, E)[b, h::2] = r["o_out"]
        w.reshape(B, 16, P, L)[b, h::2] = r["w_out"]
    _CACHE["last_result"] = res
    return out, w


# revision 48
# speedup vs baseline: 62018.4229x; 1.0020x over previous
"""Bass/Trainium2 kernel for nn_EpisodeMultiheadAttentionBlock.

Reference computation (B=4, L=4096, E=256, Q=2048):
    x  = key[:, -Q:]
    q  = rope(x @ Wq.T + bq, idx[:, -Q:]);  k = rope(key @ Wk.T + bk, idx)
    v  = key @ Wv.T + bv
    s  = (q / sqrt(E)) @ k.T  with causal mask (bottom Q rows of triu)
    w  = softmax(s)                            -> output 2
    y  = (w @ v) @ Wo.T + bo
    r  = sigmoid(x@Wxr.T + y@Wyr.T); z = sigmoid(x@Wxz.T + y@Wyz.T)
    hh = tanh((r*x)@Wxg.T + y@Wyg.T)
    out = (1-z)*x + z*hh                       -> output 1

Sharding: 8 cores = 4 batches x 2 query-interleavings. Core (b, h) owns
query 128-row blocks g = 2m+h (m=0..7) of batch b. The interleaving makes
the causal key extent per local block m uniform across cores: E(m) = 18+2m
key blocks (even-parity cores carry one fully-masked pad block), so one
SPMD program serves all 8 cores; the parity-dependent boundary mask is a
tiny per-core input added into the score PSUM via an identity matmul.

Layouts: projections/gating run in "T layout" (feature dim on partitions)
so every matmul's operands arrive in the orientation the next matmul
consumes. RoPE's pair rotation becomes a between-subtile elementwise op by
permuting the feature dim of Wq/Wk to [evens | odds] host-side (the
permutation cancels inside q.k). Scores are computed naturally
([query-part, key-free]) so the softmax row-sum falls out of the scalar
engine's exp accum_out and w DMAs straight out; the attention matmul's
j-on-partitions operand is built with PE transposes of the exp'd tiles.
"""

import sys

sys.path.insert(0, "/opt/trn_rl_repo")

import numpy as np
import ml_dtypes

import concourse.bass as bass
import concourse.tile as tile
from concourse import bacc, mybir
from concourse.bass_utils import run_bass_kernel_spmd
from concourse.masks import make_identity

F32 = mybir.dt.float32
F32R = mybir.dt.float32r
BF16 = mybir.dt.bfloat16
BF16_NP = np.dtype(ml_dtypes.bfloat16)

B, L, E, Q = 4, 4096, 256, 2048
P = 128
NM = 8              # query 128-blocks per core
NEG = -30000.0      # additive mask; exp(NEG/16) underflows to exactly 0.0
SCALE = 1.0 / 16.0  # 1/sqrt(E)

IdF = mybir.ActivationFunctionType.Identity
ExpF = mybir.ActivationFunctionType.Exp
SigF = mybir.ActivationFunctionType.Sigmoid
TanhF = mybir.ActivationFunctionType.Tanh

_CACHE = {}


def _ext_blocks(m):
    """Causal key extent (in 128-col blocks) for local query block m."""
    return 18 + 2 * m


def build_nc():
    nc = bacc.Bacc("TRN2", target_bir_lowering=False, debug=False,
                   enable_asserts=False, num_devices=8)

    io = {}
    io["keyT"] = nc.dram_tensor("keyT", [P, 2, L], F32,
                                kind="ExternalInput").ap()
    io["xT_in"] = nc.dram_tensor("xT_in", [P, 2, Q // 2], F32,
                                 kind="ExternalInput").ap()
    io["cosT"] = nc.dram_tensor("cosT", [P, L], BF16,
                                kind="ExternalInput").ap()
    io["sinT"] = nc.dram_tensor("sinT", [P, L], BF16,
                                kind="ExternalInput").ap()
    io["qcs_in"] = nc.dram_tensor("qcs_in", [P, 2, Q // 2], BF16,
                                  kind="ExternalInput").ap()
    io["wts"] = nc.dram_tensor("wts", [P, 2, 10, E], F32,
                               kind="ExternalInput").ap()
    io["mtail"] = nc.dram_tensor("mtail", [P, 2 * P], BF16,
                                 kind="ExternalInput").ap()
    io["biasv"] = nc.dram_tensor("biasv", [P, 2, 6], F32,
                                 kind="ExternalInput").ap()
    io["bvbc"] = nc.dram_tensor("bvbc", [P, E], F32,
                                kind="ExternalInput").ap()
    io["w_out"] = nc.dram_tensor("w_out", [NM, P, L], F32,
                                 kind="ExternalOutput").ap()
    io["o_out"] = nc.dram_tensor("o_out", [NM, P, E], F32,
                                 kind="ExternalOutput").ap()

    with tile.TileContext(nc) as tc:
        _emit(nc, tc, io)
    nc.compile()
    return nc


def _emit(nc, tc, io):
    from contextlib import ExitStack

    with ExitStack() as ctx:
        const = ctx.enter_context(tc.tile_pool(name="const", bufs=1))
        big = ctx.enter_context(tc.tile_pool(name="big", bufs=1))
        # single PSUM pool layout (8 banks total):
        #   mmps [P,1024] x2bufs = 4 banks, tpps [P,512] x2 = 2,
        #   atps [P,256] x2 = 2
        mmps = ctx.enter_context(tc.tile_pool(name="mmps", bufs=2,
                                              space="PSUM"))
        tpps = ctx.enter_context(tc.tile_pool(name="tpps", bufs=2,
                                              space="PSUM"))
        atps = ctx.enter_context(tc.tile_pool(name="atps", bufs=2,
                                              space="PSUM"))

        def mm_tile(cols=1024):
            return mmps.tile([P, 1024], F32, name="mmt", tag="mm")[:, :cols]

        # ---- q path first: it gates the first score matmuls ----------
        wnames = ["wq", "wk", "wv", "wo", "wxr", "wyr", "wxz", "wyz",
                  "wxg", "wyg"]
        biasv = const.tile([P, 2, 6], F32)
        nc.sync.dma_start(biasv[:], io["biasv"])
        wts = const.tile([P, 2, 10, E], F32R)
        v = big.tile([P, L // P, E], F32R)         # 32KB/part
        xTr = big.tile([P, 2, Q // 2], F32R)       # 8KB (f32r-rounded x)

        with tc.tile_pool(name="rotp", bufs=1) as rotp:
            kTrot = rotp.tile([P, 2, L], BF16)     # 16KB
            qTrot = rotp.tile([P, 2, Q // 2], BF16)

            with tc.tile_pool(name="stgp", bufs=1) as stgp, \
                 tc.tile_pool(name="keyp", bufs=1) as keyp, \
                 tc.tile_pool(name="csp", bufs=1) as csp, \
                 tc.tile_pool(name="ropet", bufs=2) as ropet:
                stg = stgp.tile([P, 2, 10, E], F32, name="stg", tag="stg")
                xs = keyp.tile([P, 2, Q // 2], F32, name="xs", tag="ks",
                               bufs=2)
                nc.sync.dma_start(xs[:], io["xT_in"])
                nc.sync.dma_start(stg[:, :, :1, :], io["wts"][:, :, :1, :])
                nc.vector.tensor_copy(wts[:, :, :1, :], stg[:, :, :1, :])
                nc.vector.tensor_copy(xTr[:], xs[:])
                qcs = csp.tile([P, 2, Q // 2], BF16)
                nc.sync.dma_start(qcs[:], io["qcs_in"])
                nc.sync.dma_start(stg[:, :, 1:3, :], io["wts"][:, :, 1:3, :])
                nc.vector.tensor_copy(wts[:, :, 1:3, :], stg[:, :, 1:3, :])
                wt = {n: wts[:, :, i, :] for i, n in enumerate(wnames)}

                def project_rot(dst, src_rhs, wn_, cos_ap, sin_ap, bcol,
                                n, cb=None, CW=1024):
                    """dst[:,0,:] = a*c - b*s ; dst[:,1,:] = a*s + b*c
                    with (a,b) = halves of (W @ src + bias)."""
                    for ci, c0 in enumerate(range(0, n, CW)):
                        cw = min(CW, n - c0)
                        pa = mm_tile(cw)
                        pb = mm_tile(cw)
                        for dh, ps in ((0, pa), (1, pb)):
                            for s in range(2):
                                for q0 in range(0, cw, 512):
                                    qw = min(512, cw - q0)
                                    nc.tensor.matmul(
                                        ps[:, q0:q0 + qw],
                                        wt[wn_][:, s, dh * P:(dh + 1) * P],
                                        src_rhs[:, s, c0 + q0:c0 + q0 + qw],
                                        start=(s == 0), stop=(s == 1))
                        ab = ropet.tile([P, 2, 1024], BF16,
                                        name="ab", tag="ab")[:, :, :cw]
                        nc.scalar.activation(ab[:, 0, :], pa[:], IdF,
                                             bias=biasv[:, 0, bcol:bcol + 1])
                        nc.scalar.activation(ab[:, 1, :], pb[:], IdF,
                                             bias=biasv[:, 1, bcol:bcol + 1])
                        c_ap = cos_ap[:, c0:c0 + cw]
                        s_ap = sin_ap[:, c0:c0 + cw]
                        t1 = ropet.tile([P, 1024], BF16, name="t1",
                                        tag="t1", bufs=1)[:, :cw]
                        t2 = ropet.tile([P, 1024], BF16, name="t2",
                                        tag="t2", bufs=1)[:, :cw]
                        nc.vector.tensor_mul(t1[:], ab[:, 0, :], c_ap)
                        nc.vector.tensor_mul(t2[:], ab[:, 1, :], s_ap)
                        nc.vector.tensor_tensor(dst[:, 0, c0:c0 + cw],
                                                t1[:], t2[:],
                                                mybir.AluOpType.subtract)
                        t3 = ropet.tile([P, 1024], BF16, name="t3",
                                        tag="t3", bufs=2)[:, :cw]
                        t4 = ropet.tile([P, 1024], BF16, name="t4",
                                        tag="t4", bufs=2)[:, :cw]
                        nc.gpsimd.tensor_mul(t3[:], ab[:, 0, :], s_ap)
                        nc.vector.tensor_mul(t4[:], ab[:, 1, :], c_ap)
                        nc.vector.tensor_tensor(dst[:, 1, c0:c0 + cw],
                                                t3[:], t4[:],
                                                mybir.AluOpType.add)
                        if cb is not None:
                            cb(ci)

                # q projection + rope (only needs xs/wq/qcs -- starts fast)
                project_rot(qTrot, xTr, "wq", qcs[:, 0, :], qcs[:, 1, :],
                            0, Q // 2)

                # keyT load + round, chunk 0 first
                keyT = keyp.tile([P, 2, L], F32R)  # 32KB
                for ci, c0 in enumerate(range(0, L, 1024)):
                    ks = keyp.tile([P, 2, 1024], F32, name="ks", tag="ks",
                                   bufs=2)
                    nc.sync.dma_start(ks[:], io["keyT"][:, :, c0:c0 + 1024])
                    nc.gpsimd.tensor_copy(keyT[:, :, c0:c0 + 1024], ks[:])
                cosT = csp.tile([P, L], BF16)
                sinT = csp.tile([P, L], BF16)
                nc.sync.dma_start(cosT[:], io["cosT"])
                nc.sync.dma_start(sinT[:], io["sinT"])

                # k projection + rope, with v-projection interleaved
                # (v group j4 covers keyT cols [512*j4, 512*j4+512) --
                #  exactly inside rope chunk ci = j4 // 2)
                def v_groups(ci):
                    for j4 in (2 * ci, 2 * ci + 1):
                        vps = mm_tile().rearrange("p (j d) -> p j d", d=E)
                        for jj in range(4):
                            J = j4 * 4 + jj
                            for s in range(2):
                                nc.tensor.matmul(
                                    vps[:, jj, :],
                                    keyT[:, s, J * P:(J + 1) * P],
                                    wt["wv"][:, s, :],
                                    start=(s == 0), stop=(s == 1))
                        dst = v[:, j4 * 4:(j4 + 1) * 4, :]
                        if j4 % 2 == 0:
                            nc.vector.tensor_copy(dst, vps[:])
                        else:
                            nc.scalar.copy(dst, vps[:])

                project_rot(kTrot, keyT, "wk", cosT, sinT, 1, L,
                            cb=v_groups)

                # remaining consts + gating weights (off critical path)
                ident = const.tile([P, P], F32)
                make_identity(nc, ident)
                identr = const.tile([P, P], F32R)
                nc.vector.tensor_copy(identr[:], ident[:])
                identb = const.tile([P, P], BF16)
                make_identity(nc, identb)
                mtail = const.tile([P, 2 * P], BF16)
                nc.sync.dma_start(mtail[:], io["mtail"])
                bvbc = const.tile([P, E], F32)
                nc.sync.dma_start(bvbc[:], io["bvbc"])
                nc.sync.dma_start(stg[:, :, 3:, :], io["wts"][:, :, 3:, :])
                nc.scalar.copy(wts[:, :, 3:, :], stg[:, :, 3:, :])

            # ---- main attention loop ---------------------------------
            with tc.tile_pool(name="expp", bufs=8) as expp, \
                 tc.tile_pool(name="wnp", bufs=4) as wnp, \
                 tc.tile_pool(name="wtp", bufs=6) as wtp, \
                 tc.tile_pool(name="zp", bufs=1) as zp, \
                 tc.tile_pool(name="gatep", bufs=1) as gatep, \
                 tc.tile_pool(name="smallp", bufs=2) as smallp:
                zeros = zp.tile([P, 1792], F32)
                nc.vector.memset(zeros[:], 0.0)
                attn = gatep.tile([P, NM, E], F32, name="attn")
                recips = gatep.tile([P, NM], F32, name="recips")

                def mm_Th(dst_sb, terms, bcol, act):
                    """dst[:,dh,:] = act(sum W.T-lhsT @ rhs[roff:] + b).
                    Sigmoid is computed as 0.5*tanh(t/2)+0.5 (biasv[bcol]
                    holds b/2) so the scalar engine never leaves the exp
                    table set -- avoids ~2.7us ACT table switches."""
                    for dh in range(2):
                        pr = atps.tile([P, 512], F32, name="pr",
                                       tag="aps")
                        ng = len(terms)
                        for gi, (wn_, rhs, roff) in enumerate(terms):
                            for s in range(2):
                                nc.tensor.matmul(
                                    pr[:],
                                    wt[wn_][:, s, dh * P:(dh + 1) * P],
                                    rhs[:, s, roff:roff + 512],
                                    start=(gi == 0 and s == 0),
                                    stop=(gi == ng - 1 and s == 1))
                        if act is SigF:
                            tmp = gatep.tile([P, 512], F32, name="sgt",
                                             tag="sgt", bufs=2)
                            nc.scalar.activation(tmp[:], pr[:], TanhF,
                                                 bias=biasv[:, dh,
                                                            bcol:bcol + 1],
                                                 scale=0.5)
                            nc.vector.tensor_scalar(dst_sb[:, dh, :],
                                                    tmp[:], 0.5, 0.5,
                                                    mybir.AluOpType.mult,
                                                    mybir.AluOpType.add)
                        else:
                            nc.scalar.activation(dst_sb[:, dh, :], pr[:],
                                                 act,
                                                 bias=biasv[:, dh,
                                                            bcol:bcol + 1])

                def emit_gate_half(half):
                    """GRU gating for query blocks [4*half, 4*half+4)."""
                    i0 = half * 512
                    attnT = gatep.tile([P, 2, 512], F32R, name="attnT",
                                       tag="attnT")
                    for dh in range(2):
                        tp = tpps.tile([P, 512], F32, name="tpg", tag="tp")
                        for mm in range(4):
                            m_ = half * 4 + mm
                            nc.tensor.transpose(
                                tp[:, mm * P:(mm + 1) * P],
                                attn[:, m_, dh * P:(dh + 1) * P],
                                ident[:])
                        if dh == 0:
                            nc.vector.tensor_copy(attnT[:, dh, :], tp[:])
                        else:
                            nc.scalar.copy(attnT[:, dh, :], tp[:])
                    yT = gatep.tile([P, 2, 512], F32R, name="yT", tag="yT")
                    mm_Th(yT, [("wo", attnT, 0)], 5, IdF)
                    rT = gatep.tile([P, 2, 512], F32, name="rT", tag="rT")
                    mm_Th(rT, [("wxr", xTr, i0), ("wyr", yT, 0)], 2, SigF)
                    zT = gatep.tile([P, 2, 512], F32, name="zT", tag="zT")
                    mm_Th(zT, [("wxz", xTr, i0), ("wyz", yT, 0)], 3, SigF)
                    xf = xTr.bitcast(F32)[:, :, i0:i0 + 512]
                    gx = gatep.tile([P, 2, 512], F32R, name="gx", tag="gx")
                    for dh in range(2):
                        nc.vector.tensor_mul(gx[:, dh, :], rT[:, dh, :],
                                             xf[:, dh, :])
                    hT = gatep.tile([P, 2, 512], F32, name="hT", tag="hT")
                    mm_Th(hT, [("wyg", yT, 0), ("wxg", gx, 0)], 4, TanhF)
                    outT = gatep.tile([P, 2, 512], F32, name="outT",
                                      tag="outT")
                    for dh in range(2):
                        d1 = gatep.tile([P, 512], F32, name="d1", tag="d1",
                                        bufs=2)
                        nc.vector.tensor_tensor(d1[:], hT[:, dh, :],
                                                xf[:, dh, :],
                                                mybir.AluOpType.subtract)
                        nc.vector.tensor_mul(d1[:], zT[:, dh, :], d1[:])
                        nc.vector.tensor_add(outT[:, dh, :], xf[:, dh, :],
                                             d1[:])
                    onat = gatep.tile([P, 4, E], F32, name="onat",
                                      tag="onat")
                    for dh in range(2):
                        tp = tpps.tile([P, 512], F32, name="tpo", tag="tp")
                        for mm in range(4):
                            nc.tensor.transpose(
                                tp[:, mm * P:(mm + 1) * P],
                                outT[:, dh, mm * P:(mm + 1) * P],
                                ident[:])
                        for mm in range(4):
                            dst = onat[:, mm, dh * P:(dh + 1) * P]
                            src = tp[:, mm * P:(mm + 1) * P]
                            if dh == 0:
                                nc.vector.tensor_copy(dst, src)
                            else:
                                nc.scalar.copy(dst, src)
                    nc.sync.dma_start(
                        io["o_out"].rearrange("m p e -> p m e")
                        [:, half * 4:half * 4 + 4, :], onat[:])

                for mi, m in enumerate([0, 1, 7, 6, 5, 4, 3, 2]):
                    EB = _ext_blocks(m)
                    ext = EB * P
                    nch = (ext + 1023) // 1024
                    parts = smallp.tile([P, 4], F32, name="parts",
                                        tag="parts")
                    aps = atps.tile([P, E], F32, name="aps", tag="aps")
                    exp_tiles = []
                    jdone = 0
                    for c in range(nch):
                        c0 = c * 1024
                        cw = min(1024, ext - c0)
                        sps = mm_tile(cw)
                        for q0 in range(0, cw, 512):
                            qw = min(512, cw - q0)
                            last = (c == nch - 1) and (q0 + qw == cw)
                            for s in range(2):
                                nc.tensor.matmul(
                                    sps[:, q0:q0 + qw],
                                    qTrot[:, s, m * P:(m + 1) * P],
                                    kTrot[:, s, c0 + q0:c0 + q0 + qw],
                                    start=(s == 0),
                                    stop=(s == 1) and not last)
                            if last:
                                nc.tensor.matmul(
                                    sps[:, cw - 2 * P:cw], identb[:],
                                    mtail[:], start=False, stop=True,
                                    skip_group_check=True)
                        ex = expp.tile([P, 1024], F32R, name="ex",
                                       tag="ex")[:, :cw]
                        nc.scalar.activation(ex[:], sps[:], ExpF, bias=0.0,
                                             scale=SCALE,
                                             accum_out=parts[:, c:c + 1])
                        exp_tiles.append((ex, c0, cw))
                        for g0 in range(0, cw, 512):
                            gw = min(512, cw - g0)
                            nblk = gw // P
                            tp = tpps.tile([P, 512], F32R, name="tp",
                                           tag="tp")[:, :gw]
                            for jj in range(nblk):
                                nc.tensor.transpose(
                                    tp[:, jj * P:(jj + 1) * P],
                                    ex[:, g0 + jj * P:g0 + (jj + 1) * P],
                                    identr[:])
                            wTs = wtp.tile([P, 512], F32R, name="wTs",
                                           tag="wTs")[:, :gw]
                            if (jdone // 4) % 2 == 0:
                                nc.vector.tensor_copy(wTs[:], tp[:])
                            else:
                                nc.scalar.copy(wTs[:], tp[:])
                            for jj in range(nblk):
                                J = jdone + jj
                                nc.tensor.matmul(
                                    aps[:],
                                    wTs[:, jj * P:(jj + 1) * P],
                                    v[:, J, :],
                                    start=(J == 0), stop=(J == EB - 1))
                            jdone += nblk
                    sig = smallp.tile([P, 1], F32, name="sig", tag="sig")
                    nc.vector.tensor_reduce(sig[:], parts[:, :nch],
                                            mybir.AxisListType.X,
                                            mybir.AluOpType.add)
                    nc.vector.reciprocal(recips[:, m:m + 1], sig[:])
                    for wi, (ex, c0, cw) in enumerate(exp_tiles):
                        wn = wnp.tile([P, 1024], F32, name="wn",
                                      tag="wn")[:, :cw]
                        weng = nc.vector if wi % 2 == 0 else nc.gpsimd
                        weng.tensor_scalar_mul(wn[:], ex.bitcast(F32),
                                               recips[:, m:m + 1])
                        deng = nc.sync if wi % 2 == 0 else nc.gpsimd
                        deng.dma_start(io["w_out"][m, :, c0:c0 + cw],
                                       wn[:])
                    if ext < L:
                        nc.sync.dma_start(io["w_out"][m, :, ext:L],
                                          zeros[:, :L - ext])
                    nc.vector.tensor_scalar_mul(attn[:, m, :], aps[:],
                                                recips[:, m:m + 1])
                    nc.vector.tensor_add(attn[:, m, :], attn[:, m, :],
                                         bvbc[:])
                    if mi == 5:
                        emit_gate_half(1)
                emit_gate_half(0)



# ======================================================================
# host side
# ======================================================================

def _prep_inputs(key, key_index, weights, biases):
    """Build the 8 per-core input dicts."""
    perm = np.concatenate([np.arange(0, E, 2), np.arange(1, E, 2)])
    inv = (1.0 / (np.float32(10000.0) **
                  (np.arange(0, E, 2).astype(np.float32) / np.float32(E))))
    inv = inv.astype(np.float32)

    stair = np.where(np.arange(P)[None, :] <= np.arange(P)[:, None],
                     np.float32(0.0), np.float32(NEG))
    mt = {
        0: np.concatenate([stair, np.full((P, P), NEG, np.float32)],
                          axis=1).astype(BF16_NP),
        1: np.concatenate([np.zeros((P, P), np.float32), stair],
                          axis=1).astype(BF16_NP),
    }

    wq, wk, wv, wo, wxr, wyr, wxz, wyz, wxg, wyg = weights
    bq, bk, bv, bo, bxr, byr, bxz, byz, bxg, byg = biases

    def lhsT(w):
        return np.ascontiguousarray(w.T).reshape(2, P, E)

    wts = np.stack([lhsT(wq[perm]), lhsT(wk[perm]), lhsT(wv), lhsT(wo),
                    lhsT(wxr), lhsT(wyr), lhsT(wxz), lhsT(wyz), lhsT(wxg),
                    lhsT(wyg)])                     # [10, 2, P, E]
    wts = np.ascontiguousarray(wts.transpose(2, 1, 0, 3))  # [P, 2, 10, E]
    biasv = np.ascontiguousarray(
        np.stack([bq[perm], bk[perm], 0.5 * (bxr + byr), 0.5 * (bxz + byz), bxg + byg, bo],
                 axis=1).reshape(2, P, 6).transpose(1, 0, 2)
        ).astype(np.float32)                        # [P, 2, 6]
    bvbc = np.broadcast_to(bv, (P, E)).astype(np.float32).copy()

    in_maps = []
    for b in range(B):
        keyT = np.ascontiguousarray(
            key[b].T.reshape(2, P, L).transpose(1, 0, 2))   # [P, 2, L]
        ang = key_index[b].astype(np.float32)[None, :] * inv[:, None]
        cosT = np.cos(ang).astype(BF16_NP)
        sinT = np.sin(ang).astype(BF16_NP)
        for h in range(2):
            qcols = (L - Q) + np.arange(Q).reshape(16, P)[h::2].reshape(-1)
            xT = np.ascontiguousarray(
                key[b][qcols].T.reshape(2, P, Q // 2).transpose(1, 0, 2))
            qcs = np.ascontiguousarray(
                np.stack([cosT[:, qcols],
                          sinT[:, qcols]]).transpose(1, 0, 2))
            in_maps.append(dict(
                keyT=keyT, cosT=cosT, sinT=sinT, mtail=np.asarray(mt[h]),
                biasv=biasv, bvbc=bvbc, xT_in=xT, qcs_in=qcs, wts=wts))
    return in_maps


def kernel(key, key_index, Wq, bq, Wk, bk, Wv, bv, Wo, bo,
           Wxr, bxr, Wyr, byr, Wxz, bxz, Wyz, byz, Wxg, bxg, Wyg, byg,
           query_length, **extra):
    key = np.asarray(key, np.float32)
    key_index = np.asarray(key_index)
    assert int(query_length) == Q and key.shape == (B, L, E)

    if "nc" not in _CACHE:
        _CACHE["nc"] = build_nc()
    nc = _CACHE["nc"]

    weights = [np.asarray(w, np.float32) for w in
               (Wq, Wk, Wv, Wo, Wxr, Wyr, Wxz, Wyz, Wxg, Wyg)]
    biases = [np.asarray(x, np.float32) for x in
              (bq, bk, bv, bo, bxr, byr, bxz, byz, bxg, byg)]
    in_maps = _prep_inputs(key, key_index, weights, biases)

    res = run_bass_kernel_spmd(nc, in_maps, core_ids=list(range(8)),
                               **_CACHE.get("run_kwargs", {}))
    out = np.empty((B, Q, E), np.float32)
    w = np.empty((B, Q, L), np.float32)
    for core in range(8):
        b, h = divmod(core, 2)
        r = res.results[core]
        out.reshape(B, 16, P, E)[b, h::2] = r["o_out"]
        w.reshape(B, 16, P, L)[b, h::2] = r["w_out"]
    _CACHE["last_result"] = res
    return out, w


# revision 52
# speedup vs baseline: 63019.2817x; 1.0161x over previous
"""Bass/Trainium2 kernel for nn_EpisodeMultiheadAttentionBlock.

Reference computation (B=4, L=4096, E=256, Q=2048):
    x  = key[:, -Q:]
    q  = rope(x @ Wq.T + bq, idx[:, -Q:]);  k = rope(key @ Wk.T + bk, idx)
    v  = key @ Wv.T + bv
    s  = (q / sqrt(E)) @ k.T  with causal mask (bottom Q rows of triu)
    w  = softmax(s)                            -> output 2
    y  = (w @ v) @ Wo.T + bo
    r  = sigmoid(x@Wxr.T + y@Wyr.T); z = sigmoid(x@Wxz.T + y@Wyz.T)
    hh = tanh((r*x)@Wxg.T + y@Wyg.T)
    out = (1-z)*x + z*hh                       -> output 1

Sharding: 8 cores = 4 batches x 2 query-interleavings. Core (b, h) owns
query 128-row blocks g = 2m+h (m=0..7) of batch b. The interleaving makes
the causal key extent per local block m uniform across cores: E(m) = 18+2m
key blocks (even-parity cores carry one fully-masked pad block), so one
SPMD program serves all 8 cores; the parity-dependent boundary mask is a
tiny per-core input added into the score PSUM via an identity matmul.

Layouts: projections/gating run in "T layout" (feature dim on partitions)
so every matmul's operands arrive in the orientation the next matmul
consumes. RoPE's pair rotation becomes a between-subtile elementwise op by
permuting the feature dim of Wq/Wk to [evens | odds] host-side (the
permutation cancels inside q.k). Scores are computed naturally
([query-part, key-free]) so the softmax row-sum falls out of the scalar
engine's exp accum_out and w DMAs straight out; the attention matmul's
j-on-partitions operand is built with PE transposes of the exp'd tiles.
"""

import sys

sys.path.insert(0, "/opt/trn_rl_repo")

import numpy as np
import ml_dtypes

import concourse.bass as bass
import concourse.tile as tile
from concourse import bacc, mybir
from concourse.bass_utils import run_bass_kernel_spmd
from concourse.masks import make_identity

F32 = mybir.dt.float32
F32R = mybir.dt.float32r
BF16 = mybir.dt.bfloat16
BF16_NP = np.dtype(ml_dtypes.bfloat16)

B, L, E, Q = 4, 4096, 256, 2048
P = 128
NM = 8              # query 128-blocks per core
NEG = -30000.0      # additive mask; exp(NEG/16) underflows to exactly 0.0
SCALE = 1.0 / 16.0  # 1/sqrt(E)

IdF = mybir.ActivationFunctionType.Identity
ExpF = mybir.ActivationFunctionType.Exp
SigF = mybir.ActivationFunctionType.Sigmoid
TanhF = mybir.ActivationFunctionType.Tanh

_CACHE = {}


def _ext_blocks(m):
    """Causal key extent (in 128-col blocks) for local query block m."""
    return 18 + 2 * m


def build_nc():
    nc = bacc.Bacc("TRN2", target_bir_lowering=False, debug=False,
                   enable_asserts=False, num_devices=8)

    io = {}
    io["keyT"] = nc.dram_tensor("keyT", [P, 2, L], F32,
                                kind="ExternalInput").ap()
    io["xT_in"] = nc.dram_tensor("xT_in", [P, 2, Q // 2], F32,
                                 kind="ExternalInput").ap()
    io["cosT"] = nc.dram_tensor("cosT", [P, L], BF16,
                                kind="ExternalInput").ap()
    io["sinT"] = nc.dram_tensor("sinT", [P, L], BF16,
                                kind="ExternalInput").ap()
    io["qcs_in"] = nc.dram_tensor("qcs_in", [P, 2, Q // 2], BF16,
                                  kind="ExternalInput").ap()
    io["wts"] = nc.dram_tensor("wts", [P, 2, 10, E], F32,
                               kind="ExternalInput").ap()
    io["mtail"] = nc.dram_tensor("mtail", [P, 2 * P], BF16,
                                 kind="ExternalInput").ap()
    io["biasv"] = nc.dram_tensor("biasv", [P, 2, 6], F32,
                                 kind="ExternalInput").ap()
    io["bvbc"] = nc.dram_tensor("bvbc", [P, E], F32,
                                kind="ExternalInput").ap()
    io["w_out"] = nc.dram_tensor("w_out", [NM, P, L], F32,
                                 kind="ExternalOutput").ap()
    io["o_out"] = nc.dram_tensor("o_out", [NM, P, E], F32,
                                 kind="ExternalOutput").ap()

    with tile.TileContext(nc) as tc:
        _emit(nc, tc, io)
    nc.compile()
    return nc


def _emit(nc, tc, io):
    from contextlib import ExitStack

    with ExitStack() as ctx:
        const = ctx.enter_context(tc.tile_pool(name="const", bufs=1))
        big = ctx.enter_context(tc.tile_pool(name="big", bufs=1))
        # single PSUM pool layout (8 banks total):
        #   mmps [P,1024] x2bufs = 4 banks, tpps [P,512] x2 = 2,
        #   atps [P,256] x2 = 2
        mmps = ctx.enter_context(tc.tile_pool(name="mmps", bufs=2,
                                              space="PSUM"))
        tpps = ctx.enter_context(tc.tile_pool(name="tpps", bufs=2,
                                              space="PSUM"))
        atps = ctx.enter_context(tc.tile_pool(name="atps", bufs=2,
                                              space="PSUM"))

        def mm_tile(cols=1024):
            return mmps.tile([P, 1024], F32, name="mmt", tag="mm")[:, :cols]

        # ---- q path first: it gates the first score matmuls ----------
        wnames = ["wq", "wk", "wv", "wo", "wxr", "wyr", "wxz", "wyz",
                  "wxg", "wyg"]
        biasv = const.tile([P, 2, 6], F32)
        nc.sync.dma_start(biasv[:], io["biasv"])
        wts = const.tile([P, 2, 10, E], F32R)
        v = big.tile([P, L // P, E], F32R)         # 32KB/part
        xTr = big.tile([P, 2, Q // 2], F32R)       # 8KB (f32r-rounded x)

        with tc.tile_pool(name="rotp", bufs=1) as rotp:
            kTrot = rotp.tile([P, 2, L], BF16)     # 16KB
            qTrot = rotp.tile([P, 2, Q // 2], BF16)

            with tc.tile_pool(name="stgp", bufs=1) as stgp, \
                 tc.tile_pool(name="keyp", bufs=1) as keyp, \
                 tc.tile_pool(name="csp", bufs=1) as csp, \
                 tc.tile_pool(name="ropet", bufs=2) as ropet:
                stg = stgp.tile([P, 2, 10, E], F32, name="stg", tag="stg")
                xs = keyp.tile([P, 2, Q // 2], F32, name="xs", tag="ks",
                               bufs=2)
                nc.sync.dma_start(xs[:], io["xT_in"])
                nc.sync.dma_start(stg[:, :, :1, :], io["wts"][:, :, :1, :])
                nc.vector.tensor_copy(wts[:, :, :1, :], stg[:, :, :1, :])
                nc.vector.tensor_copy(xTr[:], xs[:])
                qcs = csp.tile([P, 2, Q // 2], BF16)
                nc.sync.dma_start(qcs[:], io["qcs_in"])
                nc.sync.dma_start(stg[:, :, 1:3, :], io["wts"][:, :, 1:3, :])
                nc.vector.tensor_copy(wts[:, :, 1:3, :], stg[:, :, 1:3, :])
                wt = {n: wts[:, :, i, :] for i, n in enumerate(wnames)}

                def project_rot(dst, src_rhs, wn_, cos_ap, sin_ap, bcol,
                                n, cb=None, CW=1024):
                    """dst[:,0,:] = a*c - b*s ; dst[:,1,:] = a*s + b*c
                    with (a,b) = halves of (W @ src + bias)."""
                    for ci, c0 in enumerate(range(0, n, CW)):
                        cw = min(CW, n - c0)
                        pa = mm_tile(cw)
                        pb = mm_tile(cw)
                        for dh, ps in ((0, pa), (1, pb)):
                            for s in range(2):
                                for q0 in range(0, cw, 512):
                                    qw = min(512, cw - q0)
                                    nc.tensor.matmul(
                                        ps[:, q0:q0 + qw],
                                        wt[wn_][:, s, dh * P:(dh + 1) * P],
                                        src_rhs[:, s, c0 + q0:c0 + q0 + qw],
                                        start=(s == 0), stop=(s == 1))
                        ab = ropet.tile([P, 2, 1024], BF16,
                                        name="ab", tag="ab")[:, :, :cw]
                        nc.scalar.activation(ab[:, 0, :], pa[:], IdF,
                                             bias=biasv[:, 0, bcol:bcol + 1])
                        nc.scalar.activation(ab[:, 1, :], pb[:], IdF,
                                             bias=biasv[:, 1, bcol:bcol + 1])
                        c_ap = cos_ap[:, c0:c0 + cw]
                        s_ap = sin_ap[:, c0:c0 + cw]
                        t1 = ropet.tile([P, 1024], BF16, name="t1",
                                        tag="t1", bufs=1)[:, :cw]
                        t2 = ropet.tile([P, 1024], BF16, name="t2",
                                        tag="t2", bufs=1)[:, :cw]
                        nc.vector.tensor_mul(t1[:], ab[:, 0, :], c_ap)
                        nc.vector.tensor_mul(t2[:], ab[:, 1, :], s_ap)
                        nc.vector.tensor_tensor(dst[:, 0, c0:c0 + cw],
                                                t1[:], t2[:],
                                                mybir.AluOpType.subtract)
                        t3 = ropet.tile([P, 1024], BF16, name="t3",
                                        tag="t3", bufs=2)[:, :cw]
                        t4 = ropet.tile([P, 1024], BF16, name="t4",
                                        tag="t4", bufs=2)[:, :cw]
                        nc.gpsimd.tensor_mul(t3[:], ab[:, 0, :], s_ap)
                        nc.vector.tensor_mul(t4[:], ab[:, 1, :], c_ap)
                        nc.vector.tensor_tensor(dst[:, 1, c0:c0 + cw],
                                                t3[:], t4[:],
                                                mybir.AluOpType.add)
                        if cb is not None:
                            cb(ci)

                # q projection + rope (only needs xs/wq/qcs -- starts fast)
                project_rot(qTrot, xTr, "wq", qcs[:, 0, :], qcs[:, 1, :],
                            0, Q // 2)

                # keyT load + round, chunk 0 first
                keyT = keyp.tile([P, 2, L], F32R)  # 32KB
                for ci, c0 in enumerate(range(0, L, 1024)):
                    ks = keyp.tile([P, 2, 1024], F32, name="ks", tag="ks",
                                   bufs=2)
                    nc.sync.dma_start(ks[:], io["keyT"][:, :, c0:c0 + 1024])
                    nc.gpsimd.tensor_copy(keyT[:, :, c0:c0 + 1024], ks[:])
                cosT = csp.tile([P, L], BF16)
                sinT = csp.tile([P, L], BF16)
                nc.sync.dma_start(cosT[:], io["cosT"])
                nc.sync.dma_start(sinT[:], io["sinT"])

                # k projection + rope, with v-projection interleaved
                # (v group j4 covers keyT cols [512*j4, 512*j4+512) --
                #  exactly inside rope chunk ci = j4 // 2)
                def v_groups(ci):
                    for j4 in (2 * ci, 2 * ci + 1):
                        vps = mm_tile().rearrange("p (j d) -> p j d", d=E)
                        for jj in range(4):
                            J = j4 * 4 + jj
                            for s in range(2):
                                nc.tensor.matmul(
                                    vps[:, jj, :],
                                    keyT[:, s, J * P:(J + 1) * P],
                                    wt["wv"][:, s, :],
                                    start=(s == 0), stop=(s == 1))
                        dst = v[:, j4 * 4:(j4 + 1) * 4, :]
                        if j4 % 2 == 0:
                            nc.vector.tensor_copy(dst, vps[:])
                        else:
                            nc.scalar.copy(dst, vps[:])

                project_rot(kTrot, keyT, "wk", cosT, sinT, 1, L,
                            cb=v_groups)

                # remaining consts + gating weights (off critical path)
                ident = const.tile([P, P], F32)
                make_identity(nc, ident)
                identr = const.tile([P, P], F32R)
                nc.vector.tensor_copy(identr[:], ident[:])
                identb = const.tile([P, P], BF16)
                make_identity(nc, identb)
                mtail = const.tile([P, 2 * P], BF16)
                nc.sync.dma_start(mtail[:], io["mtail"])
                bvbc = const.tile([P, E], F32)
                nc.sync.dma_start(bvbc[:], io["bvbc"])
                nc.sync.dma_start(stg[:, :, 3:, :], io["wts"][:, :, 3:, :])
                nc.scalar.copy(wts[:, :, 3:, :], stg[:, :, 3:, :])

            # ---- main attention loop ---------------------------------
            with tc.tile_pool(name="expp", bufs=8) as expp, \
                 tc.tile_pool(name="wnp", bufs=4) as wnp, \
                 tc.tile_pool(name="wtp", bufs=6) as wtp, \
                 tc.tile_pool(name="zp", bufs=1) as zp, \
                 tc.tile_pool(name="gatep", bufs=1) as gatep, \
                 tc.tile_pool(name="smallp", bufs=2) as smallp:
                zeros = zp.tile([P, 1792], F32)
                nc.vector.memset(zeros[:], 0.0)
                attn = gatep.tile([P, NM, E], F32, name="attn")
                recips = gatep.tile([P, NM], F32, name="recips")

                def mm_Th(dst_sb, terms, bcol, act):
                    """dst[:,dh,:] = act(sum W.T-lhsT @ rhs[roff:] + b).
                    Sigmoid is computed as 0.5*tanh(t/2)+0.5 (biasv[bcol]
                    holds b/2) so the scalar engine never leaves the exp
                    table set -- avoids ~2.7us ACT table switches."""
                    for dh in range(2):
                        pr = atps.tile([P, 512], F32, name="pr",
                                       tag="aps")
                        ng = len(terms)
                        for gi, (wn_, rhs, roff) in enumerate(terms):
                            for s in range(2):
                                nc.tensor.matmul(
                                    pr[:],
                                    wt[wn_][:, s, dh * P:(dh + 1) * P],
                                    rhs[:, s, roff:roff + 512],
                                    start=(gi == 0 and s == 0),
                                    stop=(gi == ng - 1 and s == 1))
                        if act is SigF:
                            tmp = gatep.tile([P, 512], F32, name="sgt",
                                             tag="sgt", bufs=2)
                            nc.scalar.activation(tmp[:], pr[:], TanhF,
                                                 bias=biasv[:, dh,
                                                            bcol:bcol + 1],
                                                 scale=0.5)
                            nc.vector.tensor_scalar(dst_sb[:, dh, :],
                                                    tmp[:], 0.5, 0.5,
                                                    mybir.AluOpType.mult,
                                                    mybir.AluOpType.add)
                        else:
                            nc.scalar.activation(dst_sb[:, dh, :], pr[:],
                                                 act,
                                                 bias=biasv[:, dh,
                                                            bcol:bcol + 1])

                gstate = {}

                def emit_gate_A(half):
                    """attn -> attnT, y = attn @ Wo.T (T layout)."""
                    attnT = gatep.tile([P, 2, 512], F32R, name="attnT",
                                       tag="attnT")
                    for dh in range(2):
                        tp = tpps.tile([P, 512], F32, name="tpg", tag="tp")
                        for mm in range(4):
                            m_ = half * 4 + mm
                            nc.tensor.transpose(
                                tp[:, mm * P:(mm + 1) * P],
                                attn[:, m_, dh * P:(dh + 1) * P],
                                ident[:])
                        if dh == 0:
                            nc.vector.tensor_copy(attnT[:, dh, :], tp[:])
                        else:
                            nc.scalar.copy(attnT[:, dh, :], tp[:])
                    yT = gatep.tile([P, 2, 512], F32R, name="yT", tag="yT")
                    mm_Th(yT, [("wo", attnT, 0)], 5, IdF)
                    gstate[half] = {"yT": yT}

                def emit_gate_B(half):
                    """r, z gates + r*x."""
                    i0 = half * 512
                    yT = gstate[half]["yT"]
                    rT = gatep.tile([P, 2, 512], F32, name="rT", tag="rT")
                    mm_Th(rT, [("wxr", xTr, i0), ("wyr", yT, 0)], 2, SigF)
                    zT = gatep.tile([P, 2, 512], F32, name="zT", tag="zT")
                    mm_Th(zT, [("wxz", xTr, i0), ("wyz", yT, 0)], 3, SigF)
                    xf = xTr.bitcast(F32)[:, :, i0:i0 + 512]
                    gx = gatep.tile([P, 2, 512], F32R, name="gx", tag="gx")
                    for dh in range(2):
                        nc.vector.tensor_mul(gx[:, dh, :], rT[:, dh, :],
                                             xf[:, dh, :])
                    gstate[half].update(zT=zT, gx=gx, xf=xf)

                def emit_gate_C(half):
                    """h gate, output blend, transpose back, DMA."""
                    st = gstate[half]
                    yT, zT, gx, xf = st["yT"], st["zT"], st["gx"], st["xf"]
                    hT = gatep.tile([P, 2, 512], F32, name="hT", tag="hT")
                    mm_Th(hT, [("wyg", yT, 0), ("wxg", gx, 0)], 4, TanhF)
                    outT = gatep.tile([P, 2, 512], F32, name="outT",
                                      tag="outT")
                    for dh in range(2):
                        d1 = gatep.tile([P, 512], F32, name="d1", tag="d1",
                                        bufs=2)
                        nc.vector.tensor_tensor(d1[:], hT[:, dh, :],
                                                xf[:, dh, :],
                                                mybir.AluOpType.subtract)
                        nc.vector.tensor_mul(d1[:], zT[:, dh, :], d1[:])
                        nc.vector.tensor_add(outT[:, dh, :], xf[:, dh, :],
                                             d1[:])
                    onat = gatep.tile([P, 4, E], F32, name="onat",
                                      tag="onat")
                    for dh in range(2):
                        tp = tpps.tile([P, 512], F32, name="tpo", tag="tp")
                        for mm in range(4):
                            nc.tensor.transpose(
                                tp[:, mm * P:(mm + 1) * P],
                                outT[:, dh, mm * P:(mm + 1) * P],
                                ident[:])
                        for mm in range(4):
                            dst = onat[:, mm, dh * P:(dh + 1) * P]
                            src = tp[:, mm * P:(mm + 1) * P]
                            if dh == 0:
                                nc.vector.tensor_copy(dst, src)
                            else:
                                nc.scalar.copy(dst, src)
                    nc.sync.dma_start(
                        io["o_out"].rearrange("m p e -> p m e")
                        [:, half * 4:half * 4 + 4, :], onat[:])

                def emit_gate_half(half):
                    emit_gate_A(half)
                    emit_gate_B(half)
                    emit_gate_C(half)

                for mi, m in enumerate([0, 1, 3, 2, 7, 6, 5, 4]):
                    EB = _ext_blocks(m)
                    ext = EB * P
                    nch = (ext + 1023) // 1024
                    parts = smallp.tile([P, 4], F32, name="parts",
                                        tag="parts")
                    aps = atps.tile([P, E], F32, name="aps", tag="aps")
                    exp_tiles = []
                    jdone = 0
                    for c in range(nch):
                        c0 = c * 1024
                        cw = min(1024, ext - c0)
                        sps = mm_tile(cw)
                        for q0 in range(0, cw, 512):
                            qw = min(512, cw - q0)
                            last = (c == nch - 1) and (q0 + qw == cw)
                            for s in range(2):
                                nc.tensor.matmul(
                                    sps[:, q0:q0 + qw],
                                    qTrot[:, s, m * P:(m + 1) * P],
                                    kTrot[:, s, c0 + q0:c0 + q0 + qw],
                                    start=(s == 0),
                                    stop=(s == 1) and not last)
                            if last:
                                nc.tensor.matmul(
                                    sps[:, cw - 2 * P:cw], identb[:],
                                    mtail[:], start=False, stop=True,
                                    skip_group_check=True)
                        ex = expp.tile([P, 1024], F32R, name="ex",
                                       tag="ex")[:, :cw]
                        nc.scalar.activation(ex[:], sps[:], ExpF, bias=0.0,
                                             scale=SCALE,
                                             accum_out=parts[:, c:c + 1])
                        exp_tiles.append((ex, c0, cw))
                        for g0 in range(0, cw, 512):
                            gw = min(512, cw - g0)
                            nblk = gw // P
                            tp = tpps.tile([P, 512], F32R, name="tp",
                                           tag="tp")[:, :gw]
                            for jj in range(nblk):
                                nc.tensor.transpose(
                                    tp[:, jj * P:(jj + 1) * P],
                                    ex[:, g0 + jj * P:g0 + (jj + 1) * P],
                                    identr[:])
                            wTs = wtp.tile([P, 512], F32R, name="wTs",
                                           tag="wTs")[:, :gw]
                            if (jdone // 4) % 2 == 0:
                                nc.vector.tensor_copy(wTs[:], tp[:])
                            else:
                                nc.scalar.copy(wTs[:], tp[:])
                            for jj in range(nblk):
                                J = jdone + jj
                                nc.tensor.matmul(
                                    aps[:],
                                    wTs[:, jj * P:(jj + 1) * P],
                                    v[:, J, :],
                                    start=(J == 0), stop=(J == EB - 1))
                            jdone += nblk
                    sig = smallp.tile([P, 1], F32, name="sig", tag="sig")
                    nc.vector.tensor_reduce(sig[:], parts[:, :nch],
                                            mybir.AxisListType.X,
                                            mybir.AluOpType.add)
                    nc.vector.reciprocal(recips[:, m:m + 1], sig[:])
                    for wi, (ex, c0, cw) in enumerate(exp_tiles):
                        wn = wnp.tile([P, 1024], F32, name="wn",
                                      tag="wn")[:, :cw]
                        weng = nc.vector if wi % 2 == 0 else nc.gpsimd
                        weng.tensor_scalar_mul(wn[:], ex.bitcast(F32),
                                               recips[:, m:m + 1])
                        deng = nc.sync if wi % 2 == 0 else nc.gpsimd
                        deng.dma_start(io["w_out"][m, :, c0:c0 + cw],
                                       wn[:])
                    if ext < L:
                        nc.sync.dma_start(io["w_out"][m, :, ext:L],
                                          zeros[:, :L - ext])
                    nc.vector.tensor_scalar_mul(attn[:, m, :], aps[:],
                                                recips[:, m:m + 1])
                    nc.vector.tensor_add(attn[:, m, :], attn[:, m, :],
                                         bvbc[:])
                    if mi == 3:
                        emit_gate_A(0)
                    elif mi == 4:
                        emit_gate_B(0)
                    elif mi == 5:
                        emit_gate_C(0)
                emit_gate_half(1)



# ======================================================================
# host side
# ======================================================================

def _prep_inputs(key, key_index, weights, biases):
    """Build the 8 per-core input dicts."""
    perm = np.concatenate([np.arange(0, E, 2), np.arange(1, E, 2)])
    inv = (1.0 / (np.float32(10000.0) **
                  (np.arange(0, E, 2).astype(np.float32) / np.float32(E))))
    inv = inv.astype(np.float32)

    stair = np.where(np.arange(P)[None, :] <= np.arange(P)[:, None],
                     np.float32(0.0), np.float32(NEG))
    mt = {
        0: np.concatenate([stair, np.full((P, P), NEG, np.float32)],
                          axis=1).astype(BF16_NP),
        1: np.concatenate([np.zeros((P, P), np.float32), stair],
                          axis=1).astype(BF16_NP),
    }

    wq, wk, wv, wo, wxr, wyr, wxz, wyz, wxg, wyg = weights
    bq, bk, bv, bo, bxr, byr, bxz, byz, bxg, byg = biases

    def lhsT(w):
        return np.ascontiguousarray(w.T).reshape(2, P, E)

    wts = np.stack([lhsT(wq[perm]), lhsT(wk[perm]), lhsT(wv), lhsT(wo),
                    lhsT(wxr), lhsT(wyr), lhsT(wxz), lhsT(wyz), lhsT(wxg),
                    lhsT(wyg)])                     # [10, 2, P, E]
    wts = np.ascontiguousarray(wts.transpose(2, 1, 0, 3))  # [P, 2, 10, E]
    biasv = np.ascontiguousarray(
        np.stack([bq[perm], bk[perm], 0.5 * (bxr + byr), 0.5 * (bxz + byz), bxg + byg, bo],
                 axis=1).reshape(2, P, 6).transpose(1, 0, 2)
        ).astype(np.float32)                        # [P, 2, 6]
    bvbc = np.broadcast_to(bv, (P, E)).astype(np.float32).copy()

    in_maps = []
    for b in range(B):
        keyT = np.ascontiguousarray(
            key[b].T.reshape(2, P, L).transpose(1, 0, 2))   # [P, 2, L]
        ang = key_index[b].astype(np.float32)[None, :] * inv[:, None]
        cosT = np.cos(ang).astype(BF16_NP)
        sinT = np.sin(ang).astype(BF16_NP)
        for h in range(2):
            qcols = (L - Q) + np.arange(Q).reshape(16, P)[h::2].reshape(-1)
            xT = np.ascontiguousarray(
                key[b][qcols].T.reshape(2, P, Q // 2).transpose(1, 0, 2))
            qcs = np.ascontiguousarray(
                np.stack([cosT[:, qcols],
                          sinT[:, qcols]]).transpose(1, 0, 2))
            in_maps.append(dict(
                keyT=keyT, cosT=cosT, sinT=sinT, mtail=np.asarray(mt[h]),
                biasv=biasv, bvbc=bvbc, xT_in=xT, qcs_in=qcs, wts=wts))
    return in_maps


def kernel(key, key_index, Wq, bq, Wk, bk, Wv, bv, Wo, bo,
           Wxr, bxr, Wyr, byr, Wxz, bxz, Wyz, byz, Wxg, bxg, Wyg, byg,
           query_length, **extra):
    key = np.asarray(key, np.float32)
    key_index = np.asarray(key_index)
    assert int(query_length) == Q and key.shape == (B, L, E)

    if "nc" not in _CACHE:
        _CACHE["nc"] = build_nc()
    nc = _CACHE["nc"]

    weights = [np.asarray(w, np.float32) for w in
               (Wq, Wk, Wv, Wo, Wxr, Wyr, Wxz, Wyz, Wxg, Wyg)]
    biases = [np.asarray(x, np.float32) for x in
              (bq, bk, bv, bo, bxr, byr, bxz, byz, bxg, byg)]
    in_maps = _prep_inputs(key, key_index, weights, biases)

    res = run_bass_kernel_spmd(nc, in_maps, core_ids=list(range(8)),
                               **_CACHE.get("run_kwargs", {}))
    out = np.empty((B, Q, E), np.float32)
    w = np.empty((B, Q, L), np.float32)
    for core in range(8):
        b, h = divmod(core, 2)
        r = res.results[core]
        out.reshape(B, 16, P, E)[b, h::2] = r["o_out"]
        w.reshape(B, 16, P, L)[b, h::2] = r["w_out"]
    _CACHE["last_result"] = res
    return out, w


# revision 54
# speedup vs baseline: 64636.5023x; 1.0257x over previous
"""Bass/Trainium2 kernel for nn_EpisodeMultiheadAttentionBlock.

Reference computation (B=4, L=4096, E=256, Q=2048):
    x  = key[:, -Q:]
    q  = rope(x @ Wq.T + bq, idx[:, -Q:]);  k = rope(key @ Wk.T + bk, idx)
    v  = key @ Wv.T + bv
    s  = (q / sqrt(E)) @ k.T  with causal mask (bottom Q rows of triu)
    w  = softmax(s)                            -> output 2
    y  = (w @ v) @ Wo.T + bo
    r  = sigmoid(x@Wxr.T + y@Wyr.T); z = sigmoid(x@Wxz.T + y@Wyz.T)
    hh = tanh((r*x)@Wxg.T + y@Wyg.T)
    out = (1-z)*x + z*hh                       -> output 1

Sharding: 8 cores = 4 batches x 2 query-interleavings. Core (b, h) owns
query 128-row blocks g = 2m+h (m=0..7) of batch b. The interleaving makes
the causal key extent per local block m uniform across cores: E(m) = 18+2m
key blocks (even-parity cores carry one fully-masked pad block), so one
SPMD program serves all 8 cores; the parity-dependent boundary mask is a
tiny per-core input added into the score PSUM via an identity matmul.

Layouts: projections/gating run in "T layout" (feature dim on partitions)
so every matmul's operands arrive in the orientation the next matmul
consumes. RoPE's pair rotation becomes a between-subtile elementwise op by
permuting the feature dim of Wq/Wk to [evens | odds] host-side (the
permutation cancels inside q.k). Scores are computed naturally
([query-part, key-free]) so the softmax row-sum falls out of the scalar
engine's exp accum_out and w DMAs straight out; the attention matmul's
j-on-partitions operand is built with PE transposes of the exp'd tiles.
"""

import sys

sys.path.insert(0, "/opt/trn_rl_repo")

import numpy as np
import ml_dtypes

import concourse.bass as bass
import concourse.tile as tile
from concourse import bacc, mybir
from concourse.bass_utils import run_bass_kernel_spmd
from concourse.masks import make_identity

F32 = mybir.dt.float32
F32R = mybir.dt.float32r
BF16 = mybir.dt.bfloat16
BF16_NP = np.dtype(ml_dtypes.bfloat16)

B, L, E, Q = 4, 4096, 256, 2048
P = 128
NM = 8              # query 128-blocks per core
NEG = -30000.0      # additive mask; exp(NEG/16) underflows to exactly 0.0
SCALE = 1.0 / 16.0  # 1/sqrt(E)

IdF = mybir.ActivationFunctionType.Identity
ExpF = mybir.ActivationFunctionType.Exp
SigF = mybir.ActivationFunctionType.Sigmoid
TanhF = mybir.ActivationFunctionType.Tanh

_CACHE = {}


def _ext_blocks(m):
    """Causal key extent (in 128-col blocks) for local query block m."""
    return 18 + 2 * m


def build_nc():
    nc = bacc.Bacc("TRN2", target_bir_lowering=False, debug=False,
                   enable_asserts=False, num_devices=8)

    io = {}
    io["keyT"] = nc.dram_tensor("keyT", [P, 2, L], F32,
                                kind="ExternalInput").ap()
    io["xT_in"] = nc.dram_tensor("xT_in", [P, 2, Q // 2], F32,
                                 kind="ExternalInput").ap()
    io["cosT"] = nc.dram_tensor("cosT", [P, L], BF16,
                                kind="ExternalInput").ap()
    io["sinT"] = nc.dram_tensor("sinT", [P, L], BF16,
                                kind="ExternalInput").ap()
    io["qcs_in"] = nc.dram_tensor("qcs_in", [P, 2, Q // 2], BF16,
                                  kind="ExternalInput").ap()
    io["wts"] = nc.dram_tensor("wts", [P, 2, 10, E], F32,
                               kind="ExternalInput").ap()
    io["mtail"] = nc.dram_tensor("mtail", [P, 2 * P], BF16,
                                 kind="ExternalInput").ap()
    io["biasv"] = nc.dram_tensor("biasv", [P, 2, 6], F32,
                                 kind="ExternalInput").ap()
    io["bvbc"] = nc.dram_tensor("bvbc", [P, E], F32,
                                kind="ExternalInput").ap()
    io["w_out"] = nc.dram_tensor("w_out", [NM, P, L], F32,
                                 kind="ExternalOutput").ap()
    io["o_out"] = nc.dram_tensor("o_out", [NM, P, E], F32,
                                 kind="ExternalOutput").ap()

    with tile.TileContext(nc) as tc:
        _emit(nc, tc, io)
    nc.compile()
    return nc


def _emit(nc, tc, io):
    from contextlib import ExitStack

    with ExitStack() as ctx:
        const = ctx.enter_context(tc.tile_pool(name="const", bufs=1))
        big = ctx.enter_context(tc.tile_pool(name="big", bufs=1))
        # single PSUM pool layout (8 banks total):
        #   mmps [P,1024] x2bufs = 4 banks, tpps [P,512] x2 = 2,
        #   atps [P,256] x2 = 2
        mmps = ctx.enter_context(tc.tile_pool(name="mmps", bufs=2,
                                              space="PSUM"))
        tpps = ctx.enter_context(tc.tile_pool(name="tpps", bufs=2,
                                              space="PSUM"))
        atps = ctx.enter_context(tc.tile_pool(name="atps", bufs=2,
                                              space="PSUM"))

        def mm_tile(cols=1024):
            return mmps.tile([P, 1024], F32, name="mmt", tag="mm")[:, :cols]

        # ---- q path first: it gates the first score matmuls ----------
        wnames = ["wq", "wk", "wv", "wo", "wxr", "wyr", "wxz", "wyz",
                  "wxg", "wyg"]
        biasv = const.tile([P, 2, 6], F32)
        nc.sync.dma_start(biasv[:], io["biasv"])
        wts = const.tile([P, 2, 10, E], F32R)
        v = big.tile([P, L // P, E], F32R)         # 32KB/part
        xTr = big.tile([P, 2, Q // 2], F32R)       # 8KB (f32r-rounded x)

        with tc.tile_pool(name="rotp", bufs=1) as rotp:
            kTrot = rotp.tile([P, 2, L], BF16)     # 16KB
            qTrot = rotp.tile([P, 2, Q // 2], BF16)

            with tc.tile_pool(name="stgp", bufs=1) as stgp, \
                 tc.tile_pool(name="keyp", bufs=1) as keyp, \
                 tc.tile_pool(name="csp", bufs=1) as csp, \
                 tc.tile_pool(name="ropet", bufs=2) as ropet:
                stg = stgp.tile([P, 2, 10, E], F32, name="stg", tag="stg")
                xs = keyp.tile([P, 2, Q // 2], F32, name="xs", tag="ks",
                               bufs=2)
                nc.sync.dma_start(xs[:], io["xT_in"])
                nc.sync.dma_start(stg[:, :, :1, :], io["wts"][:, :, :1, :])
                nc.vector.tensor_copy(wts[:, :, :1, :], stg[:, :, :1, :])
                nc.vector.tensor_copy(xTr[:], xs[:])
                qcs = csp.tile([P, 2, Q // 2], BF16)
                nc.sync.dma_start(qcs[:], io["qcs_in"])
                nc.sync.dma_start(stg[:, :, 1:3, :], io["wts"][:, :, 1:3, :])
                nc.vector.tensor_copy(wts[:, :, 1:3, :], stg[:, :, 1:3, :])
                wt = {n: wts[:, :, i, :] for i, n in enumerate(wnames)}

                def project_rot(dst, src_rhs, wn_, cos_ap, sin_ap, bcol,
                                n, cb=None, CW=1024):
                    """dst[:,0,:] = a*c - b*s ; dst[:,1,:] = a*s + b*c
                    with (a,b) = halves of (W @ src + bias)."""
                    for ci, c0 in enumerate(range(0, n, CW)):
                        cw = min(CW, n - c0)
                        pa = mm_tile(cw)
                        pb = mm_tile(cw)
                        for dh, ps in ((0, pa), (1, pb)):
                            for s in range(2):
                                for q0 in range(0, cw, 512):
                                    qw = min(512, cw - q0)
                                    nc.tensor.matmul(
                                        ps[:, q0:q0 + qw],
                                        wt[wn_][:, s, dh * P:(dh + 1) * P],
                                        src_rhs[:, s, c0 + q0:c0 + q0 + qw],
                                        start=(s == 0), stop=(s == 1))
                        ab = ropet.tile([P, 2, 1024], BF16,
                                        name="ab", tag="ab")[:, :, :cw]
                        nc.scalar.activation(ab[:, 0, :], pa[:], IdF,
                                             bias=biasv[:, 0, bcol:bcol + 1])
                        nc.scalar.activation(ab[:, 1, :], pb[:], IdF,
                                             bias=biasv[:, 1, bcol:bcol + 1])
                        c_ap = cos_ap[:, c0:c0 + cw]
                        s_ap = sin_ap[:, c0:c0 + cw]
                        t1 = ropet.tile([P, 1024], BF16, name="t1",
                                        tag="t1", bufs=1)[:, :cw]
                        t2 = ropet.tile([P, 1024], BF16, name="t2",
                                        tag="t2", bufs=1)[:, :cw]
                        nc.vector.tensor_mul(t1[:], ab[:, 0, :], c_ap)
                        nc.vector.tensor_mul(t2[:], ab[:, 1, :], s_ap)
                        nc.vector.tensor_tensor(dst[:, 0, c0:c0 + cw],
                                                t1[:], t2[:],
                                                mybir.AluOpType.subtract)
                        t3 = ropet.tile([P, 1024], BF16, name="t3",
                                        tag="t3", bufs=2)[:, :cw]
                        t4 = ropet.tile([P, 1024], BF16, name="t4",
                                        tag="t4", bufs=2)[:, :cw]
                        nc.gpsimd.tensor_mul(t3[:], ab[:, 0, :], s_ap)
                        nc.vector.tensor_mul(t4[:], ab[:, 1, :], c_ap)
                        nc.vector.tensor_tensor(dst[:, 1, c0:c0 + cw],
                                                t3[:], t4[:],
                                                mybir.AluOpType.add)
                        if cb is not None:
                            cb(ci)

                # q projection + rope (only needs xs/wq/qcs -- starts fast)
                project_rot(qTrot, xTr, "wq", qcs[:, 0, :], qcs[:, 1, :],
                            0, Q // 2)

                # keyT load + round, chunk 0 first
                keyT = keyp.tile([P, 2, L], F32R)  # 32KB
                for ci, c0 in enumerate(range(0, L, 1024)):
                    ks = keyp.tile([P, 2, 1024], F32, name="ks", tag="ks",
                                   bufs=2)
                    nc.sync.dma_start(ks[:], io["keyT"][:, :, c0:c0 + 1024])
                    nc.gpsimd.tensor_copy(keyT[:, :, c0:c0 + 1024], ks[:])
                cosT = csp.tile([P, L], BF16)
                sinT = csp.tile([P, L], BF16)
                nc.sync.dma_start(cosT[:], io["cosT"])
                nc.sync.dma_start(sinT[:], io["sinT"])

                # k projection + rope, with v-projection interleaved
                # (v group j4 covers keyT cols [512*j4, 512*j4+512) --
                #  exactly inside rope chunk ci = j4 // 2)
                def v_groups(ci):
                    for j4 in (2 * ci, 2 * ci + 1):
                        vps = mm_tile().rearrange("p (j d) -> p j d", d=E)
                        for jj in range(4):
                            J = j4 * 4 + jj
                            for s in range(2):
                                nc.tensor.matmul(
                                    vps[:, jj, :],
                                    keyT[:, s, J * P:(J + 1) * P],
                                    wt["wv"][:, s, :],
                                    start=(s == 0), stop=(s == 1))
                        dst = v[:, j4 * 4:(j4 + 1) * 4, :]
                        if j4 % 2 == 0:
                            nc.vector.tensor_copy(dst, vps[:])
                        else:
                            nc.scalar.copy(dst, vps[:])

                project_rot(kTrot, keyT, "wk", cosT, sinT, 1, L,
                            cb=v_groups)

                # remaining consts + gating weights (off critical path)
                ident = const.tile([P, P], F32)
                make_identity(nc, ident)
                identr = const.tile([P, P], F32R)
                nc.vector.tensor_copy(identr[:], ident[:])
                identb = const.tile([P, P], BF16)
                make_identity(nc, identb)
                mtail = const.tile([P, 2 * P], BF16)
                nc.sync.dma_start(mtail[:], io["mtail"])
                bvbc = const.tile([P, E], F32)
                nc.sync.dma_start(bvbc[:], io["bvbc"])
                nc.sync.dma_start(stg[:, :, 3:, :], io["wts"][:, :, 3:, :])
                nc.scalar.copy(wts[:, :, 3:, :], stg[:, :, 3:, :])

            # ---- main attention loop ---------------------------------
            with tc.tile_pool(name="expp", bufs=8) as expp, \
                 tc.tile_pool(name="wnp", bufs=4) as wnp, \
                 tc.tile_pool(name="wtp", bufs=6) as wtp, \
                 tc.tile_pool(name="zp", bufs=1) as zp, \
                 tc.tile_pool(name="gatep", bufs=1) as gatep, \
                 tc.tile_pool(name="smallp", bufs=2) as smallp:
                zeros = zp.tile([P, 1792], F32)
                nc.vector.memset(zeros[:], 0.0)
                attn = gatep.tile([P, NM, E], F32, name="attn")
                recips = gatep.tile([P, NM], F32, name="recips")

                def mm_Th(dst_sb, terms, bcol, act, w=512):
                    """dst[:,dh,:] = act(sum W.T-lhsT @ rhs[roff:] + b).
                    Sigmoid is computed as 0.5*tanh(t/2)+0.5 (biasv[bcol]
                    holds b/2) so the scalar engine never leaves the exp
                    table set -- avoids ~2.7us ACT table switches."""
                    for dh in range(2):
                        pr = atps.tile([P, 512], F32, name="pr",
                                       tag="aps")[:, :w]
                        ng = len(terms)
                        for gi, (wn_, rhs, roff) in enumerate(terms):
                            for s in range(2):
                                nc.tensor.matmul(
                                    pr[:],
                                    wt[wn_][:, s, dh * P:(dh + 1) * P],
                                    rhs[:, s, roff:roff + w],
                                    start=(gi == 0 and s == 0),
                                    stop=(gi == ng - 1 and s == 1))
                        if act is SigF:
                            tmp = gatep.tile([P, 512], F32, name="sgt",
                                             tag="sgt", bufs=2)[:, :w]
                            nc.scalar.activation(tmp[:], pr[:], TanhF,
                                                 bias=biasv[:, dh,
                                                            bcol:bcol + 1],
                                                 scale=0.5)
                            nc.vector.tensor_scalar(dst_sb[:, dh, :],
                                                    tmp[:], 0.5, 0.5,
                                                    mybir.AluOpType.mult,
                                                    mybir.AluOpType.add)
                        else:
                            nc.scalar.activation(dst_sb[:, dh, :], pr[:],
                                                 act,
                                                 bias=biasv[:, dh,
                                                            bcol:bcol + 1])

                gstate = {}

                def emit_gate_A(key, i0, w, ms):
                    """attn -> attnT, y = attn @ Wo.T (T layout)."""
                    nb = len(ms)
                    attnT = gatep.tile([P, 2, 512], F32R, name="attnT",
                                       tag="attnT")[:, :, :w]
                    for dh in range(2):
                        tp = tpps.tile([P, 512], F32, name="tpg",
                                       tag="tp")[:, :w]
                        for mm in range(nb):
                            nc.tensor.transpose(
                                tp[:, mm * P:(mm + 1) * P],
                                attn[:, ms[mm], dh * P:(dh + 1) * P],
                                ident[:])
                        if dh == 0:
                            nc.vector.tensor_copy(attnT[:, dh, :], tp[:])
                        else:
                            nc.scalar.copy(attnT[:, dh, :], tp[:])
                    yT = gatep.tile([P, 2, 512], F32R, name="yT",
                                    tag="yT")[:, :, :w]
                    mm_Th(yT, [("wo", attnT, 0)], 5, IdF, w)
                    gstate[key] = {"yT": yT, "i0": i0, "w": w, "ms": ms}

                def emit_gate_B(key):
                    """r, z gates + r*x."""
                    st = gstate[key]
                    yT, i0, w = st["yT"], st["i0"], st["w"]
                    rT = gatep.tile([P, 2, 512], F32, name="rT",
                                    tag="rT")[:, :, :w]
                    mm_Th(rT, [("wxr", xTr, i0), ("wyr", yT, 0)], 2, SigF, w)
                    zT = gatep.tile([P, 2, 512], F32, name="zT",
                                    tag="zT")[:, :, :w]
                    mm_Th(zT, [("wxz", xTr, i0), ("wyz", yT, 0)], 3, SigF, w)
                    xf = xTr.bitcast(F32)[:, :, i0:i0 + w]
                    gx = gatep.tile([P, 2, 512], F32R, name="gx",
                                    tag="gx")[:, :, :w]
                    for dh in range(2):
                        nc.vector.tensor_mul(gx[:, dh, :], rT[:, dh, :],
                                             xf[:, dh, :])
                    st.update(zT=zT, gx=gx, xf=xf)

                def emit_gate_C(key):
                    """h gate, output blend, transpose back, DMA."""
                    st = gstate[key]
                    yT, zT, gx, xf = st["yT"], st["zT"], st["gx"], st["xf"]
                    w, ms = st["w"], st["ms"]
                    nb = len(ms)
                    hT = gatep.tile([P, 2, 512], F32, name="hT",
                                    tag="hT")[:, :, :w]
                    mm_Th(hT, [("wyg", yT, 0), ("wxg", gx, 0)], 4, TanhF, w)
                    outT = gatep.tile([P, 2, 512], F32, name="outT",
                                      tag="outT")[:, :, :w]
                    for dh in range(2):
                        d1 = gatep.tile([P, 512], F32, name="d1", tag="d1",
                                        bufs=2)[:, :w]
                        nc.vector.tensor_tensor(d1[:], hT[:, dh, :],
                                                xf[:, dh, :],
                                                mybir.AluOpType.subtract)
                        nc.vector.tensor_mul(d1[:], zT[:, dh, :], d1[:])
                        nc.vector.tensor_add(outT[:, dh, :], xf[:, dh, :],
                                             d1[:])
                    onat = gatep.tile([P, 4, E], F32, name="onat",
                                      tag="onat")[:, :nb, :]
                    for dh in range(2):
                        tp = tpps.tile([P, 512], F32, name="tpo",
                                       tag="tp")[:, :w]
                        for mm in range(nb):
                            nc.tensor.transpose(
                                tp[:, mm * P:(mm + 1) * P],
                                outT[:, dh, mm * P:(mm + 1) * P],
                                ident[:])
                        for mm in range(nb):
                            dst = onat[:, mm, dh * P:(dh + 1) * P]
                            src = tp[:, mm * P:(mm + 1) * P]
                            if dh == 0:
                                nc.vector.tensor_copy(dst, src)
                            else:
                                nc.scalar.copy(dst, src)
                    nc.sync.dma_start(
                        io["o_out"].rearrange("m p e -> p m e")
                        [:, ms[0]:ms[0] + nb, :], onat[:])

                for mi, m in enumerate([0, 1, 3, 2, 7, 6, 5, 4]):
                    EB = _ext_blocks(m)
                    ext = EB * P
                    nch = (ext + 1023) // 1024
                    parts = smallp.tile([P, 4], F32, name="parts",
                                        tag="parts")
                    aps = atps.tile([P, E], F32, name="aps", tag="aps")
                    exp_tiles = []
                    jdone = 0
                    for c in range(nch):
                        c0 = c * 1024
                        cw = min(1024, ext - c0)
                        sps = mm_tile(cw)
                        for q0 in range(0, cw, 512):
                            qw = min(512, cw - q0)
                            last = (c == nch - 1) and (q0 + qw == cw)
                            for s in range(2):
                                nc.tensor.matmul(
                                    sps[:, q0:q0 + qw],
                                    qTrot[:, s, m * P:(m + 1) * P],
                                    kTrot[:, s, c0 + q0:c0 + q0 + qw],
                                    start=(s == 0),
                                    stop=(s == 1) and not last)
                            if last:
                                nc.tensor.matmul(
                                    sps[:, cw - 2 * P:cw], identb[:],
                                    mtail[:], start=False, stop=True,
                                    skip_group_check=True)
                        ex = expp.tile([P, 1024], F32R, name="ex",
                                       tag="ex")[:, :cw]
                        nc.scalar.activation(ex[:], sps[:], ExpF, bias=0.0,
                                             scale=SCALE,
                                             accum_out=parts[:, c:c + 1])
                        exp_tiles.append((ex, c0, cw))
                        for g0 in range(0, cw, 512):
                            gw = min(512, cw - g0)
                            nblk = gw // P
                            tp = tpps.tile([P, 512], F32R, name="tp",
                                           tag="tp")[:, :gw]
                            for jj in range(nblk):
                                nc.tensor.transpose(
                                    tp[:, jj * P:(jj + 1) * P],
                                    ex[:, g0 + jj * P:g0 + (jj + 1) * P],
                                    identr[:])
                            wTs = wtp.tile([P, 512], F32R, name="wTs",
                                           tag="wTs")[:, :gw]
                            if (jdone // 4) % 2 == 0:
                                nc.vector.tensor_copy(wTs[:], tp[:])
                            else:
                                nc.scalar.copy(wTs[:], tp[:])
                            for jj in range(nblk):
                                J = jdone + jj
                                nc.tensor.matmul(
                                    aps[:],
                                    wTs[:, jj * P:(jj + 1) * P],
                                    v[:, J, :],
                                    start=(J == 0), stop=(J == EB - 1))
                            jdone += nblk
                    sig = smallp.tile([P, 1], F32, name="sig", tag="sig")
                    nc.vector.tensor_reduce(sig[:], parts[:, :nch],
                                            mybir.AxisListType.X,
                                            mybir.AluOpType.add)
                    nc.vector.reciprocal(recips[:, m:m + 1], sig[:])
                    for wi, (ex, c0, cw) in enumerate(exp_tiles):
                        wn = wnp.tile([P, 1024], F32, name="wn",
                                      tag="wn")[:, :cw]
                        weng = nc.vector if wi % 2 == 0 else nc.gpsimd
                        weng.tensor_scalar_mul(wn[:], ex.bitcast(F32),
                                               recips[:, m:m + 1])
                        deng = nc.sync if wi % 2 == 0 else nc.gpsimd
                        deng.dma_start(io["w_out"][m, :, c0:c0 + cw],
                                       wn[:])
                    if ext < L:
                        nc.sync.dma_start(io["w_out"][m, :, ext:L],
                                          zeros[:, :L - ext])
                    nc.vector.tensor_scalar_mul(attn[:, m, :], aps[:],
                                                recips[:, m:m + 1])
                    nc.vector.tensor_add(attn[:, m, :], attn[:, m, :],
                                         bvbc[:])
                    if mi == 3:
                        emit_gate_A("g0", 0, 512, [0, 1, 2, 3])
                    elif mi == 4:
                        emit_gate_B("g0")
                    elif mi == 5:
                        emit_gate_C("g0")
                        emit_gate_A("q3", 768, 256, [6, 7])
                    elif mi == 6:
                        emit_gate_B("q3")
                    elif mi == 7:
                        emit_gate_C("q3")
                emit_gate_A("q2", 512, 256, [4, 5])
                emit_gate_B("q2")
                emit_gate_C("q2")



# ======================================================================
# host side
# ======================================================================

def _prep_inputs(key, key_index, weights, biases):
    """Build the 8 per-core input dicts."""
    perm = np.concatenate([np.arange(0, E, 2), np.arange(1, E, 2)])
    inv = (1.0 / (np.float32(10000.0) **
                  (np.arange(0, E, 2).astype(np.float32) / np.float32(E))))
    inv = inv.astype(np.float32)

    stair = np.where(np.arange(P)[None, :] <= np.arange(P)[:, None],
                     np.float32(0.0), np.float32(NEG))
    mt = {
        0: np.concatenate([stair, np.full((P, P), NEG, np.float32)],
                          axis=1).astype(BF16_NP),
        1: np.concatenate([np.zeros((P, P), np.float32), stair],
                          axis=1).astype(BF16_NP),
    }

    wq, wk, wv, wo, wxr, wyr, wxz, wyz, wxg, wyg = weights
    bq, bk, bv, bo, bxr, byr, bxz, byz, bxg, byg = biases

    def lhsT(w):
        return np.ascontiguousarray(w.T).reshape(2, P, E)

    wts = np.stack([lhsT(wq[perm]), lhsT(wk[perm]), lhsT(wv), lhsT(wo),
                    lhsT(wxr), lhsT(wyr), lhsT(wxz), lhsT(wyz), lhsT(wxg),
                    lhsT(wyg)])                     # [10, 2, P, E]
    wts = np.ascontiguousarray(wts.transpose(2, 1, 0, 3))  # [P, 2, 10, E]
    biasv = np.ascontiguousarray(
        np.stack([bq[perm], bk[perm], 0.5 * (bxr + byr), 0.5 * (bxz + byz), bxg + byg, bo],
                 axis=1).reshape(2, P, 6).transpose(1, 0, 2)
        ).astype(np.float32)                        # [P, 2, 6]
    bvbc = np.broadcast_to(bv, (P, E)).astype(np.float32).copy()

    in_maps = []
    for b in range(B):
        keyT = np.ascontiguousarray(
            key[b].T.reshape(2, P, L).transpose(1, 0, 2))   # [P, 2, L]
        ang = key_index[b].astype(np.float32)[None, :] * inv[:, None]
        cosT = np.cos(ang).astype(BF16_NP)
        sinT = np.sin(ang).astype(BF16_NP)
        for h in range(2):
            qcols = (L - Q) + np.arange(Q).reshape(16, P)[h::2].reshape(-1)
            xT = np.ascontiguousarray(
                key[b][qcols].T.reshape(2, P, Q // 2).transpose(1, 0, 2))
            qcs = np.ascontiguousarray(
                np.stack([cosT[:, qcols],
                          sinT[:, qcols]]).transpose(1, 0, 2))
            in_maps.append(dict(
                keyT=keyT, cosT=cosT, sinT=sinT, mtail=np.asarray(mt[h]),
                biasv=biasv, bvbc=bvbc, xT_in=xT, qcs_in=qcs, wts=wts))
    return in_maps


def kernel(key, key_index, Wq, bq, Wk, bk, Wv, bv, Wo, bo,
           Wxr, bxr, Wyr, byr, Wxz, bxz, Wyz, byz, Wxg, bxg, Wyg, byg,
           query_length, **extra):
    key = np.asarray(key, np.float32)
    key_index = np.asarray(key_index)
    assert int(query_length) == Q and key.shape == (B, L, E)

    if "nc" not in _CACHE:
        _CACHE["nc"] = build_nc()
    nc = _CACHE["nc"]

    weights = [np.asarray(w, np.float32) for w in
               (Wq, Wk, Wv, Wo, Wxr, Wyr, Wxz, Wyz, Wxg, Wyg)]
    biases = [np.asarray(x, np.float32) for x in
              (bq, bk, bv, bo, bxr, byr, bxz, byz, bxg, byg)]
    in_maps = _prep_inputs(key, key_index, weights, biases)

    res = run_bass_kernel_spmd(nc, in_maps, core_ids=list(range(8)),
                               **_CACHE.get("run_kwargs", {}))
    out = np.empty((B, Q, E), np.float32)
    w = np.empty((B, Q, L), np.float32)
    for core in range(8):
        b, h = divmod(core, 2)
        r = res.results[core]
        out.reshape(B, 16, P, E)[b, h::2] = r["o_out"]
        w.reshape(B, 16, P, L)[b, h::2] = r["w_out"]
    _CACHE["last_result"] = res
    return out, w


# revision 56
# speedup vs baseline: 64924.9540x; 1.0045x over previous
"""Bass/Trainium2 kernel for nn_EpisodeMultiheadAttentionBlock.

Reference computation (B=4, L=4096, E=256, Q=2048):
    x  = key[:, -Q:]
    q  = rope(x @ Wq.T + bq, idx[:, -Q:]);  k = rope(key @ Wk.T + bk, idx)
    v  = key @ Wv.T + bv
    s  = (q / sqrt(E)) @ k.T  with causal mask (bottom Q rows of triu)
    w  = softmax(s)                            -> output 2
    y  = (w @ v) @ Wo.T + bo
    r  = sigmoid(x@Wxr.T + y@Wyr.T); z = sigmoid(x@Wxz.T + y@Wyz.T)
    hh = tanh((r*x)@Wxg.T + y@Wyg.T)
    out = (1-z)*x + z*hh                       -> output 1

Sharding: 8 cores = 4 batches x 2 query-interleavings. Core (b, h) owns
query 128-row blocks g = 2m+h (m=0..7) of batch b. The interleaving makes
the causal key extent per local block m uniform across cores: E(m) = 18+2m
key blocks (even-parity cores carry one fully-masked pad block), so one
SPMD program serves all 8 cores; the parity-dependent boundary mask is a
tiny per-core input added into the score PSUM via an identity matmul.

Layouts: projections/gating run in "T layout" (feature dim on partitions)
so every matmul's operands arrive in the orientation the next matmul
consumes. RoPE's pair rotation becomes a between-subtile elementwise op by
permuting the feature dim of Wq/Wk to [evens | odds] host-side (the
permutation cancels inside q.k). Scores are computed naturally
([query-part, key-free]) so the softmax row-sum falls out of the scalar
engine's exp accum_out and w DMAs straight out; the attention matmul's
j-on-partitions operand is built with PE transposes of the exp'd tiles.
"""

import sys

sys.path.insert(0, "/opt/trn_rl_repo")

import numpy as np
import ml_dtypes

import concourse.bass as bass
import concourse.tile as tile
from concourse import bacc, mybir
from concourse.bass_utils import run_bass_kernel_spmd
from concourse.masks import make_identity

F32 = mybir.dt.float32
F32R = mybir.dt.float32r
BF16 = mybir.dt.bfloat16
BF16_NP = np.dtype(ml_dtypes.bfloat16)

B, L, E, Q = 4, 4096, 256, 2048
P = 128
NM = 8              # query 128-blocks per core
NEG = -30000.0      # additive mask; exp(NEG/16) underflows to exactly 0.0
SCALE = 1.0 / 16.0  # 1/sqrt(E)

IdF = mybir.ActivationFunctionType.Identity
ExpF = mybir.ActivationFunctionType.Exp
SigF = mybir.ActivationFunctionType.Sigmoid
TanhF = mybir.ActivationFunctionType.Tanh

_CACHE = {}


def _ext_blocks(m):
    """Causal key extent (in 128-col blocks) for local query block m."""
    return 18 + 2 * m


def build_nc():
    nc = bacc.Bacc("TRN2", target_bir_lowering=False, debug=False,
                   enable_asserts=False, num_devices=8)

    io = {}
    io["keyT"] = nc.dram_tensor("keyT", [P, 2, L], F32,
                                kind="ExternalInput").ap()
    io["xT_in"] = nc.dram_tensor("xT_in", [P, 2, Q // 2], F32,
                                 kind="ExternalInput").ap()
    io["cosT"] = nc.dram_tensor("cosT", [P, L], BF16,
                                kind="ExternalInput").ap()
    io["sinT"] = nc.dram_tensor("sinT", [P, L], BF16,
                                kind="ExternalInput").ap()
    io["qcs_in"] = nc.dram_tensor("qcs_in", [P, 2, Q // 2], BF16,
                                  kind="ExternalInput").ap()
    io["wts"] = nc.dram_tensor("wts", [P, 2, 10, E], F32,
                               kind="ExternalInput").ap()
    io["mtail"] = nc.dram_tensor("mtail", [P, 2 * P], BF16,
                                 kind="ExternalInput").ap()
    io["biasv"] = nc.dram_tensor("biasv", [P, 2, 6], F32,
                                 kind="ExternalInput").ap()
    io["bvbc"] = nc.dram_tensor("bvbc", [P, E], F32,
                                kind="ExternalInput").ap()
    io["w_out"] = nc.dram_tensor("w_out", [NM, P, L], F32,
                                 kind="ExternalOutput").ap()
    io["o_out"] = nc.dram_tensor("o_out", [NM, P, E], F32,
                                 kind="ExternalOutput").ap()

    with tile.TileContext(nc) as tc:
        _emit(nc, tc, io)
    nc.compile()
    return nc


def _emit(nc, tc, io):
    from contextlib import ExitStack

    with ExitStack() as ctx:
        const = ctx.enter_context(tc.tile_pool(name="const", bufs=1))
        big = ctx.enter_context(tc.tile_pool(name="big", bufs=1))
        # single PSUM pool layout (8 banks total):
        #   mmps [P,1024] x2bufs = 4 banks, tpps [P,512] x2 = 2,
        #   atps [P,256] x2 = 2
        mmps = ctx.enter_context(tc.tile_pool(name="mmps", bufs=2,
                                              space="PSUM"))
        tpps = ctx.enter_context(tc.tile_pool(name="tpps", bufs=2,
                                              space="PSUM"))
        atps = ctx.enter_context(tc.tile_pool(name="atps", bufs=2,
                                              space="PSUM"))

        def mm_tile(cols=1024):
            return mmps.tile([P, 1024], F32, name="mmt", tag="mm")[:, :cols]

        # ---- q path first: it gates the first score matmuls ----------
        wnames = ["wq", "wk", "wv", "wo", "wxr", "wyr", "wxz", "wyz",
                  "wxg", "wyg"]
        biasv = const.tile([P, 2, 6], F32)
        wts = const.tile([P, 2, 10, E], F32R)
        v = big.tile([P, L // P, E], F32R)         # 32KB/part
        xTr = big.tile([P, 2, Q // 2], F32R)       # 8KB (f32r-rounded x)

        with tc.tile_pool(name="rotp", bufs=1) as rotp:
            kTrot = rotp.tile([P, 2, L], BF16)     # 16KB
            qTrot = rotp.tile([P, 2, Q // 2], BF16)

            with tc.tile_pool(name="stgp", bufs=1) as stgp, \
                 tc.tile_pool(name="keyp", bufs=1) as keyp, \
                 tc.tile_pool(name="csp", bufs=1) as csp, \
                 tc.tile_pool(name="ropet", bufs=2) as ropet:
                stg = stgp.tile([P, 2, 10, E], F32, name="stg", tag="stg")
                xs = keyp.tile([P, 2, Q // 2], F32, name="xs", tag="ks",
                               bufs=2)
                nc.sync.dma_start(xs[:], io["xT_in"])
                nc.sync.dma_start(stg[:, :, :1, :], io["wts"][:, :, :1, :])
                nc.vector.tensor_copy(wts[:, :, :1, :], stg[:, :, :1, :])
                nc.vector.tensor_copy(xTr[:], xs[:])
                qcs = csp.tile([P, 2, Q // 2], BF16)
                nc.sync.dma_start(qcs[:], io["qcs_in"])
                nc.sync.dma_start(biasv[:], io["biasv"])
                nc.sync.dma_start(stg[:, :, 1:3, :], io["wts"][:, :, 1:3, :])
                nc.vector.tensor_copy(wts[:, :, 1:3, :], stg[:, :, 1:3, :])
                wt = {n: wts[:, :, i, :] for i, n in enumerate(wnames)}

                def project_rot(dst, src_rhs, wn_, cos_ap, sin_ap, bcol,
                                n, cb=None, CW=1024):
                    """dst[:,0,:] = a*c - b*s ; dst[:,1,:] = a*s + b*c
                    with (a,b) = halves of (W @ src + bias)."""
                    for ci, c0 in enumerate(range(0, n, CW)):
                        cw = min(CW, n - c0)
                        pa = mm_tile(cw)
                        pb = mm_tile(cw)
                        for dh, ps in ((0, pa), (1, pb)):
                            for s in range(2):
                                for q0 in range(0, cw, 512):
                                    qw = min(512, cw - q0)
                                    nc.tensor.matmul(
                                        ps[:, q0:q0 + qw],
                                        wt[wn_][:, s, dh * P:(dh + 1) * P],
                                        src_rhs[:, s, c0 + q0:c0 + q0 + qw],
                                        start=(s == 0), stop=(s == 1))
                        ab = ropet.tile([P, 2, 1024], BF16,
                                        name="ab", tag="ab")[:, :, :cw]
                        nc.scalar.activation(ab[:, 0, :], pa[:], IdF,
                                             bias=biasv[:, 0, bcol:bcol + 1])
                        nc.scalar.activation(ab[:, 1, :], pb[:], IdF,
                                             bias=biasv[:, 1, bcol:bcol + 1])
                        c_ap = cos_ap[:, c0:c0 + cw]
                        s_ap = sin_ap[:, c0:c0 + cw]
                        t1 = ropet.tile([P, 1024], BF16, name="t1",
                                        tag="t1", bufs=1)[:, :cw]
                        t2 = ropet.tile([P, 1024], BF16, name="t2",
                                        tag="t2", bufs=1)[:, :cw]
                        nc.vector.tensor_mul(t1[:], ab[:, 0, :], c_ap)
                        nc.vector.tensor_mul(t2[:], ab[:, 1, :], s_ap)
                        nc.vector.tensor_tensor(dst[:, 0, c0:c0 + cw],
                                                t1[:], t2[:],
                                                mybir.AluOpType.subtract)
                        t3 = ropet.tile([P, 1024], BF16, name="t3",
                                        tag="t3", bufs=2)[:, :cw]
                        t4 = ropet.tile([P, 1024], BF16, name="t4",
                                        tag="t4", bufs=2)[:, :cw]
                        nc.gpsimd.tensor_mul(t3[:], ab[:, 0, :], s_ap)
                        nc.vector.tensor_mul(t4[:], ab[:, 1, :], c_ap)
                        nc.vector.tensor_tensor(dst[:, 1, c0:c0 + cw],
                                                t3[:], t4[:],
                                                mybir.AluOpType.add)
                        if cb is not None:
                            cb(ci)

                # q projection + rope (only needs xs/wq/qcs -- starts fast)
                project_rot(qTrot, xTr, "wq", qcs[:, 0, :], qcs[:, 1, :],
                            0, Q // 2)

                # keyT load + round, chunk 0 first
                keyT = keyp.tile([P, 2, L], F32R)  # 32KB
                for ci, c0 in enumerate(range(0, L, 1024)):
                    ks = keyp.tile([P, 2, 1024], F32, name="ks", tag="ks",
                                   bufs=2)
                    nc.sync.dma_start(ks[:], io["keyT"][:, :, c0:c0 + 1024])
                    nc.gpsimd.tensor_copy(keyT[:, :, c0:c0 + 1024], ks[:])
                cosT = csp.tile([P, L], BF16)
                sinT = csp.tile([P, L], BF16)
                nc.sync.dma_start(cosT[:], io["cosT"])
                nc.sync.dma_start(sinT[:], io["sinT"])

                # k projection + rope, with v-projection interleaved
                # (v group j4 covers keyT cols [512*j4, 512*j4+512) --
                #  exactly inside rope chunk ci = j4 // 2)
                def v_groups(ci):
                    for j4 in (2 * ci, 2 * ci + 1):
                        vps = mm_tile().rearrange("p (j d) -> p j d", d=E)
                        for jj in range(4):
                            J = j4 * 4 + jj
                            for s in range(2):
                                nc.tensor.matmul(
                                    vps[:, jj, :],
                                    keyT[:, s, J * P:(J + 1) * P],
                                    wt["wv"][:, s, :],
                                    start=(s == 0), stop=(s == 1))
                        dst = v[:, j4 * 4:(j4 + 1) * 4, :]
                        if j4 % 2 == 0:
                            nc.vector.tensor_copy(dst, vps[:])
                        else:
                            nc.scalar.copy(dst, vps[:])

                project_rot(kTrot, keyT, "wk", cosT, sinT, 1, L,
                            cb=v_groups)

                # remaining consts + gating weights (off critical path)
                ident = const.tile([P, P], F32)
                make_identity(nc, ident)
                identr = const.tile([P, P], F32R)
                nc.vector.tensor_copy(identr[:], ident[:])
                identb = const.tile([P, P], BF16)
                make_identity(nc, identb)
                mtail = const.tile([P, 2 * P], BF16)
                nc.sync.dma_start(mtail[:], io["mtail"])
                bvbc = const.tile([P, E], F32)
                nc.sync.dma_start(bvbc[:], io["bvbc"])
                nc.sync.dma_start(stg[:, :, 3:, :], io["wts"][:, :, 3:, :])
                nc.scalar.copy(wts[:, :, 3:, :], stg[:, :, 3:, :])

            # ---- main attention loop ---------------------------------
            with tc.tile_pool(name="expp", bufs=8) as expp, \
                 tc.tile_pool(name="wnp", bufs=4) as wnp, \
                 tc.tile_pool(name="wtp", bufs=6) as wtp, \
                 tc.tile_pool(name="zp", bufs=1) as zp, \
                 tc.tile_pool(name="gatep", bufs=1) as gatep, \
                 tc.tile_pool(name="smallp", bufs=2) as smallp:
                zeros = zp.tile([P, 1792], F32)
                nc.vector.memset(zeros[:], 0.0)
                attn = gatep.tile([P, NM, E], F32, name="attn")
                recips = gatep.tile([P, NM], F32, name="recips")

                def mm_Th(dst_sb, terms, bcol, act, w=512):
                    """dst[:,dh,:] = act(sum W.T-lhsT @ rhs[roff:] + b).
                    Sigmoid is computed as 0.5*tanh(t/2)+0.5 (biasv[bcol]
                    holds b/2) so the scalar engine never leaves the exp
                    table set -- avoids ~2.7us ACT table switches."""
                    for dh in range(2):
                        pr = atps.tile([P, 512], F32, name="pr",
                                       tag="aps")[:, :w]
                        ng = len(terms)
                        for gi, (wn_, rhs, roff) in enumerate(terms):
                            for s in range(2):
                                nc.tensor.matmul(
                                    pr[:],
                                    wt[wn_][:, s, dh * P:(dh + 1) * P],
                                    rhs[:, s, roff:roff + w],
                                    start=(gi == 0 and s == 0),
                                    stop=(gi == ng - 1 and s == 1))
                        if act is SigF:
                            tmp = gatep.tile([P, 512], F32, name="sgt",
                                             tag="sgt", bufs=2)[:, :w]
                            nc.scalar.activation(tmp[:], pr[:], TanhF,
                                                 bias=biasv[:, dh,
                                                            bcol:bcol + 1],
                                                 scale=0.5)
                            nc.vector.tensor_scalar(dst_sb[:, dh, :],
                                                    tmp[:], 0.5, 0.5,
                                                    mybir.AluOpType.mult,
                                                    mybir.AluOpType.add)
                        else:
                            nc.scalar.activation(dst_sb[:, dh, :], pr[:],
                                                 act,
                                                 bias=biasv[:, dh,
                                                            bcol:bcol + 1])

                gstate = {}

                def emit_gate_A(key, i0, w, ms):
                    """attn -> attnT, y = attn @ Wo.T (T layout)."""
                    nb = len(ms)
                    attnT = gatep.tile([P, 2, 512], F32R, name="attnT",
                                       tag="attnT")[:, :, :w]
                    for dh in range(2):
                        tp = tpps.tile([P, 512], F32, name="tpg",
                                       tag="tp")[:, :w]
                        for mm in range(nb):
                            nc.tensor.transpose(
                                tp[:, mm * P:(mm + 1) * P],
                                attn[:, ms[mm], dh * P:(dh + 1) * P],
                                ident[:])
                        if dh == 0:
                            nc.vector.tensor_copy(attnT[:, dh, :], tp[:])
                        else:
                            nc.scalar.copy(attnT[:, dh, :], tp[:])
                    yT = gatep.tile([P, 2, 512], F32R, name="yT",
                                    tag="yT")[:, :, :w]
                    mm_Th(yT, [("wo", attnT, 0)], 5, IdF, w)
                    gstate[key] = {"yT": yT, "i0": i0, "w": w, "ms": ms}

                def emit_gate_B(key):
                    """r, z gates + r*x."""
                    st = gstate[key]
                    yT, i0, w = st["yT"], st["i0"], st["w"]
                    rT = gatep.tile([P, 2, 512], F32, name="rT",
                                    tag="rT")[:, :, :w]
                    mm_Th(rT, [("wxr", xTr, i0), ("wyr", yT, 0)], 2, SigF, w)
                    zT = gatep.tile([P, 2, 512], F32, name="zT",
                                    tag="zT")[:, :, :w]
                    mm_Th(zT, [("wxz", xTr, i0), ("wyz", yT, 0)], 3, SigF, w)
                    xf = xTr.bitcast(F32)[:, :, i0:i0 + w]
                    gx = gatep.tile([P, 2, 512], F32R, name="gx",
                                    tag="gx")[:, :, :w]
                    for dh in range(2):
                        nc.vector.tensor_mul(gx[:, dh, :], rT[:, dh, :],
                                             xf[:, dh, :])
                    st.update(zT=zT, gx=gx, xf=xf)

                def emit_gate_C(key):
                    """h gate, output blend, transpose back, DMA."""
                    st = gstate[key]
                    yT, zT, gx, xf = st["yT"], st["zT"], st["gx"], st["xf"]
                    w, ms = st["w"], st["ms"]
                    nb = len(ms)
                    hT = gatep.tile([P, 2, 512], F32, name="hT",
                                    tag="hT")[:, :, :w]
                    mm_Th(hT, [("wyg", yT, 0), ("wxg", gx, 0)], 4, TanhF, w)
                    outT = gatep.tile([P, 2, 512], F32, name="outT",
                                      tag="outT")[:, :, :w]
                    for dh in range(2):
                        d1 = gatep.tile([P, 512], F32, name="d1", tag="d1",
                                        bufs=2)[:, :w]
                        nc.vector.tensor_tensor(d1[:], hT[:, dh, :],
                                                xf[:, dh, :],
                                                mybir.AluOpType.subtract)
                        nc.vector.tensor_mul(d1[:], zT[:, dh, :], d1[:])
                        nc.vector.tensor_add(outT[:, dh, :], xf[:, dh, :],
                                             d1[:])
                    onat = gatep.tile([P, 4, E], F32, name="onat",
                                      tag="onat")[:, :nb, :]
                    for dh in range(2):
                        tp = tpps.tile([P, 512], F32, name="tpo",
                                       tag="tp")[:, :w]
                        for mm in range(nb):
                            nc.tensor.transpose(
                                tp[:, mm * P:(mm + 1) * P],
                                outT[:, dh, mm * P:(mm + 1) * P],
                                ident[:])
                        for mm in range(nb):
                            dst = onat[:, mm, dh * P:(dh + 1) * P]
                            src = tp[:, mm * P:(mm + 1) * P]
                            if dh == 0:
                                nc.vector.tensor_copy(dst, src)
                            else:
                                nc.scalar.copy(dst, src)
                    nc.sync.dma_start(
                        io["o_out"].rearrange("m p e -> p m e")
                        [:, ms[0]:ms[0] + nb, :], onat[:])

                for mi, m in enumerate([0, 1, 3, 2, 7, 6, 5, 4]):
                    EB = _ext_blocks(m)
                    ext = EB * P
                    nch = (ext + 1023) // 1024
                    parts = smallp.tile([P, 4], F32, name="parts",
                                        tag="parts")
                    aps = atps.tile([P, E], F32, name="aps", tag="aps")
                    exp_tiles = []
                    jdone = 0
                    for c in range(nch):
                        c0 = c * 1024
                        cw = min(1024, ext - c0)
                        sps = mm_tile(cw)
                        for q0 in range(0, cw, 512):
                            qw = min(512, cw - q0)
                            last = (c == nch - 1) and (q0 + qw == cw)
                            for s in range(2):
                                nc.tensor.matmul(
                                    sps[:, q0:q0 + qw],
                                    qTrot[:, s, m * P:(m + 1) * P],
                                    kTrot[:, s, c0 + q0:c0 + q0 + qw],
                                    start=(s == 0),
                                    stop=(s == 1) and not last)
                            if last:
                                nc.tensor.matmul(
                                    sps[:, cw - 2 * P:cw], identb[:],
                                    mtail[:], start=False, stop=True,
                                    skip_group_check=True)
                        ex = expp.tile([P, 1024], F32R, name="ex",
                                       tag="ex")[:, :cw]
                        nc.scalar.activation(ex[:], sps[:], ExpF, bias=0.0,
                                             scale=SCALE,
                                             accum_out=parts[:, c:c + 1])
                        exp_tiles.append((ex, c0, cw))
                        for g0 in range(0, cw, 512):
                            gw = min(512, cw - g0)
                            nblk = gw // P
                            tp = tpps.tile([P, 512], F32R, name="tp",
                                           tag="tp")[:, :gw]
                            for jj in range(nblk):
                                nc.tensor.transpose(
                                    tp[:, jj * P:(jj + 1) * P],
                                    ex[:, g0 + jj * P:g0 + (jj + 1) * P],
                                    identr[:])
                            wTs = wtp.tile([P, 512], F32R, name="wTs",
                                           tag="wTs")[:, :gw]
                            if (jdone // 4) % 2 == 0:
                                nc.vector.tensor_copy(wTs[:], tp[:])
                            else:
                                nc.scalar.copy(wTs[:], tp[:])
                            for jj in range(nblk):
                                J = jdone + jj
                                nc.tensor.matmul(
                                    aps[:],
                                    wTs[:, jj * P:(jj + 1) * P],
                                    v[:, J, :],
                                    start=(J == 0), stop=(J == EB - 1))
                            jdone += nblk
                    sig = smallp.tile([P, 1], F32, name="sig", tag="sig")
                    nc.vector.tensor_reduce(sig[:], parts[:, :nch],
                                            mybir.AxisListType.X,
                                            mybir.AluOpType.add)
                    nc.vector.reciprocal(recips[:, m:m + 1], sig[:])
                    for wi, (ex, c0, cw) in enumerate(exp_tiles):
                        wn = wnp.tile([P, 1024], F32, name="wn",
                                      tag="wn")[:, :cw]
                        weng = nc.vector if wi % 2 == 0 else nc.gpsimd
                        weng.tensor_scalar_mul(wn[:], ex.bitcast(F32),
                                               recips[:, m:m + 1])
                        deng = nc.sync if wi % 2 == 0 else nc.gpsimd
                        deng.dma_start(io["w_out"][m, :, c0:c0 + cw],
                                       wn[:])
                    if ext < L:
                        nc.sync.dma_start(io["w_out"][m, :, ext:L],
                                          zeros[:, :L - ext])
                    nc.vector.tensor_scalar_mul(attn[:, m, :], aps[:],
                                                recips[:, m:m + 1])
                    nc.vector.tensor_add(attn[:, m, :], attn[:, m, :],
                                         bvbc[:])
                    if mi == 3:
                        emit_gate_A("g0", 0, 512, [0, 1, 2, 3])
                    elif mi == 4:
                        emit_gate_B("g0")
                    elif mi == 5:
                        emit_gate_C("g0")
                        emit_gate_A("q3", 768, 256, [6, 7])
                    elif mi == 6:
                        emit_gate_B("q3")
                    elif mi == 7:
                        emit_gate_C("q3")
                emit_gate_A("q2", 512, 256, [4, 5])
                emit_gate_B("q2")
                emit_gate_C("q2")



# ======================================================================
# host side
# ======================================================================

def _prep_inputs(key, key_index, weights, biases):
    """Build the 8 per-core input dicts."""
    perm = np.concatenate([np.arange(0, E, 2), np.arange(1, E, 2)])
    inv = (1.0 / (np.float32(10000.0) **
                  (np.arange(0, E, 2).astype(np.float32) / np.float32(E))))
    inv = inv.astype(np.float32)

    stair = np.where(np.arange(P)[None, :] <= np.arange(P)[:, None],
                     np.float32(0.0), np.float32(NEG))
    mt = {
        0: np.concatenate([stair, np.full((P, P), NEG, np.float32)],
                          axis=1).astype(BF16_NP),
        1: np.concatenate([np.zeros((P, P), np.float32), stair],
                          axis=1).astype(BF16_NP),
    }

    wq, wk, wv, wo, wxr, wyr, wxz, wyz, wxg, wyg = weights
    bq, bk, bv, bo, bxr, byr, bxz, byz, bxg, byg = biases

    def lhsT(w):
        return np.ascontiguousarray(w.T).reshape(2, P, E)

    wts = np.stack([lhsT(wq[perm]), lhsT(wk[perm]), lhsT(wv), lhsT(wo),
                    lhsT(wxr), lhsT(wyr), lhsT(wxz), lhsT(wyz), lhsT(wxg),
                    lhsT(wyg)])                     # [10, 2, P, E]
    wts = np.ascontiguousarray(wts.transpose(2, 1, 0, 3))  # [P, 2, 10, E]
    biasv = np.ascontiguousarray(
        np.stack([bq[perm], bk[perm], 0.5 * (bxr + byr), 0.5 * (bxz + byz), bxg + byg, bo],
                 axis=1).reshape(2, P, 6).transpose(1, 0, 2)
        ).astype(np.float32)                        # [P, 2, 6]
    bvbc = np.broadcast_to(bv, (P, E)).astype(np.float32).copy()

    in_maps = []
    for b in range(B):
        keyT = np.ascontiguousarray(
            key[b].T.reshape(2, P, L).transpose(1, 0, 2))   # [P, 2, L]
        ang = key_index[b].astype(np.float32)[None, :] * inv[:, None]
        cosT = np.cos(ang).astype(BF16_NP)
        sinT = np.sin(ang).astype(BF16_NP)
        for h in range(2):
            qcols = (L - Q) + np.arange(Q).reshape(16, P)[h::2].reshape(-1)
            xT = np.ascontiguousarray(
                key[b][qcols].T.reshape(2, P, Q // 2).transpose(1, 0, 2))
            qcs = np.ascontiguousarray(
                np.stack([cosT[:, qcols],
                          sinT[:, qcols]]).transpose(1, 0, 2))
            in_maps.append(dict(
                keyT=keyT, cosT=cosT, sinT=sinT, mtail=np.asarray(mt[h]),
                biasv=biasv, bvbc=bvbc, xT_in=xT, qcs_in=qcs, wts=wts))
    return in_maps


def kernel(key, key_index, Wq, bq, Wk, bk, Wv, bv, Wo, bo,
           Wxr, bxr, Wyr, byr, Wxz, bxz, Wyz, byz, Wxg, bxg, Wyg, byg,
           query_length, **extra):
    key = np.asarray(key, np.float32)
    key_index = np.asarray(key_index)
    assert int(query_length) == Q and key.shape == (B, L, E)

    if "nc" not in _CACHE:
        _CACHE["nc"] = build_nc()
    nc = _CACHE["nc"]

    weights = [np.asarray(w, np.float32) for w in
               (Wq, Wk, Wv, Wo, Wxr, Wyr, Wxz, Wyz, Wxg, Wyg)]
    biases = [np.asarray(x, np.float32) for x in
              (bq, bk, bv, bo, bxr, byr, bxz, byz, bxg, byg)]
    in_maps = _prep_inputs(key, key_index, weights, biases)

    res = run_bass_kernel_spmd(nc, in_maps, core_ids=list(range(8)),
                               **_CACHE.get("run_kwargs", {}))
    out = np.empty((B, Q, E), np.float32)
    w = np.empty((B, Q, L), np.float32)
    for core in range(8):
        b, h = divmod(core, 2)
        r = res.results[core]
        out.reshape(B, 16, P, E)[b, h::2] = r["o_out"]
        w.reshape(B, 16, P, L)[b, h::2] = r["w_out"]
    _CACHE["last_result"] = res
    return out, w
